# revision 31
# baseline (speedup 1.0000x reference)
"""Trainium2 Bass kernel for nn_Action_Decoder (GAT-based action decoder).

v3 strategy (8 NeuronCores, pure data-parallel over batch):
  - B=4096 sharded 8 x 512 samples/core; weights replicated; 4 tiles of
    128 samples on the partition dim.
  - Gather restructure: host packs a [BS*36, 7*128] bf16 block table
    (rows for (sample, substation) = 6 node rows + 1 sub row, built with
    sample-INDEPENDENT indexing from the fixed [36,6] element table).
    The device gathers ONE 1792-byte block per sample: 2 dma_gather ops
    of 256 idxs each (vs 8 ops / 3584 descriptors in v2) -> SWDGE
    descriptor generation drops ~7x and blocks land directly in x^T
    layout ([feature, k, sample]).
  - idx DMA issued first on the sync queue; consts spread across
    vector/tensor/scalar queues so nothing delays the gather.
  - Layer-1 on PE as v2 (obs folded via W_proj@W1a, shared obs+sub part
    broadcast over the 6 nodes with an identity-stationary matmul,
    fused transpose+e via the trident matmul).
  - Tail engine rebalance: ALL PSUM evictions + exps on Scalar, e_raw /
    Z1 / relu / one alpha*h product row on GpSimd, DVE keeps the
    irreducible tensor-tensor chain (apply + folds + elu-combine + W2).
  - elu fused: elu(x)+1 = min(exp(x),1) + relu(x), one scalar Exp + one
    DVE scalar_tensor_tensor; W2 dot via one mult + one axis-reduce.
"""

import os
import sys

import numpy as np

for _p in ("/root/.axon_site", "/root/.axon_site/_ro/trn_rl_repo",
           "/root/.axon_site/_ro/pypackages", "/opt/trn_rl_repo", "/opt/pypackages"):
    if os.path.isdir(_p) and _p not in sys.path:
        sys.path.append(_p)

import ml_dtypes

import concourse.bass as bass
import concourse.tile as tile
from concourse import bacc
from concourse import mybir
from concourse.bass_utils import run_bass_kernel_spmd

# Problem dims
B, N, S, K, H, OBS = 4096, 177, 36, 6, 128, 500
HEADS, FH = 4, 32
NCORES = 8
BS = B // NCORES          # 512 samples per core
NT = BS // 128            # 4 tiles of 128 samples
OBS_PAD = 512             # pad 500 -> 512
RB = 7                    # rows per (sample, substation) block

F32 = mybir.dt.float32
BF16 = mybir.dt.bfloat16
I16 = mybir.dt.int16
AX = mybir.AxisListType
OP = mybir.AluOpType
ACT = mybir.ActivationFunctionType

LRELU_SLOPE = 0.2
# GpSimd does SWDGE gathers ONLY. Measured: any gpsimd elementwise op
# running concurrently with DVE slows BOTH ~3-40x (shared SBUF port),
# and using gpsimd tensor ops forces a Pool ucode lib swap (~12us).


def build_graph(scalars):
    as2 = float(scalars["a_src2"])
    ad2 = float(scalars["a_dst2"])
    b2 = float(scalars["b2"])
    c2 = float(scalars["c2"])
    ce_nz = bool(scalars["ce_nz"])

    nc = bacc.Bacc(num_swdge_queues=2)

    table7 = nc.declare_dram_parameter("table7", [BS * S, RB * H], BF16,
                                       isOutput=False)
    idx2 = nc.declare_dram_parameter("idx2", [128, 32], I16, isOutput=False)
    obs_T = nc.declare_dram_parameter("obs_T", [128, 4, BS], BF16, isOutput=False)
    wfold = nc.declare_dram_parameter("wfold", [128, 4, H], BF16, isOutput=False)
    w1bc = nc.declare_dram_parameter("w1bc", [H, 2, H], BF16, isOutput=False)
    trident = nc.declare_dram_parameter("trident", [H, 136], BF16, isOutput=False)
    identd = nc.declare_dram_parameter("identd", [H, H], BF16, isOutput=False)
    biash = nc.declare_dram_parameter("biash", [H, 1], F32, isOutput=False)
    ce = nc.declare_dram_parameter("ce", [1, 48], F32, isOutput=False)
    w2r = nc.declare_dram_parameter("w2r", [1, H], BF16, isOutput=False)
    out_ext = nc.declare_dram_parameter("out", [BS, K], F32, isOutput=True)

    with tile.TileContext(nc) as tc:
        with (
            tc.tile_pool(name="consts", bufs=1) as consts,
            tc.tile_pool(name="gat", bufs=1) as gat,
            tc.tile_pool(name="ht", bufs=2) as htp,
            tc.tile_pool(name="work", bufs=1) as work,
            tc.tile_pool(name="psS", bufs=1, space="PSUM") as psS,
            tc.tile_pool(name="psH", bufs=2, space="PSUM") as psH,
            tc.tile_pool(name="psT", bufs=1, space="PSUM") as psT,
        ):
            # ---- idx first (the gathers' only dependency), on sync ----
            idx_sb = consts.tile([128, 32], I16)
            nc.sync.dma_start(out=idx_sb, in_=idx2[:, :])

            # ---- consts spread over the sync + scalar queues ----
            obs_sb = consts.tile([128, 4, BS], BF16)
            nc.sync.dma_start(out=obs_sb[:, 0:2, :], in_=obs_T[:, 0:2, :])
            nc.scalar.dma_start(out=obs_sb[:, 2:4, :], in_=obs_T[:, 2:4, :])
            wfold_sb = consts.tile([128, 4, 128], BF16)
            nc.scalar.dma_start(out=wfold_sb[:, :, :], in_=wfold[:, :, :])
            w1bc_sb = consts.tile([128, 2, 128], BF16)
            nc.sync.dma_start(out=w1bc_sb[:, :, :], in_=w1bc[:, :, :])
            tri_sb = consts.tile([128, 136], BF16)
            nc.sync.dma_start(out=tri_sb, in_=trident[:, :])
            ident_sb = consts.tile([128, 128], BF16)
            nc.sync.dma_start(out=ident_sb, in_=identd[:, :])
            biash_sb = consts.tile([128, 1], F32)
            nc.scalar.dma_start(out=biash_sb, in_=biash[:, :])
            w2_sb = consts.tile([128, 128], BF16)
            nc.scalar.dma_start(
                out=w2_sb,
                in_=bass.AP(tensor=w2r, offset=0, ap=[[0, 128], [1, 128]]),
            )
            if ce_nz:
                ce_sb = consts.tile([128, 48], F32)
                nc.scalar.dma_start(
                    out=ce_sb,
                    in_=bass.AP(tensor=ce, offset=0, ap=[[0, 128], [1, 48]]),
                )

            # ---- gathers: one 1792B block per sample, 256 idx per pair.
            # The first SWDGE instruction absorbs the Pool ucode lib-load
            # latency (~11.5us after MODIFY_POOL_CONFIG). ----
            gs = []
            for p in range(2):
                g = gat.tile([128, RB, 256], BF16, tag=f"g{p}")
                nc.gpsimd.dma_gather(
                    out_ap=g[:, :, :],
                    in_ap=table7[:, :],
                    idxs_ap=idx_sb[:, p * 16:(p + 1) * 16],
                    num_idxs=256, num_idxs_reg=256, elem_size=RB * H,
                    transpose=True, queue_num=p,
                )
                gs.append(g)

            # ---- shared part: obs (4 accumulating chunks over all 512
            # samples) + per-tile sub row. First psS write must be the only
            # start=True on that bank. ----
            shF = psS.tile([128, 512], F32)
            for c in range(4):
                nc.tensor.matmul(shF[:, :], wfold_sb[:, c, :], obs_sb[:, c, :],
                                 start=(c == 0), stop=False)

            sh_sb = work.tile([128, 4, 128], BF16, tag="sh")
            hbL = work.tile([128, NT, 768], BF16, tag="hbL")
            e_sd = work.tile([128, NT, 48], F32, tag="esd")
            e4 = e_sd[:, :, :].rearrange("p t (k s h) -> p t k s h", s=2, h=4)
            e_raw = work.tile([128, NT, 144], BF16, tag="eraw")
            e_rawf = e_raw[:, :, :].rearrange("p t x -> p (t x)")
            e_lrf = work.tile([128, NT * 144], BF16, tag="elr")
            E1 = work.tile([128, NT * 144], F32, tag="E1")
            Z1 = work.tile([128, 96], F32, tag="Z1")
            rZ1 = work.tile([128, 96], F32, tag="rZ1")
            al = work.tile([128, NT, 144], BF16, tag="al")
            prod = work.tile([128, 24, 768], BF16, tag="prod")
            f1 = work.tile([128, 24, 384], BF16, tag="f1")
            f2 = work.tile([128, 24, 128], BF16, tag="f2")
            attn = work.tile([128, 24, 128], BF16, tag="attn")
            minat = work.tile([128, 24, 128], BF16, tag="minat")
            expm = work.tile([128, 24, 128], BF16, tag="expm")
            relux = work.tile([128, 24, 128], BF16, tag="relux")
            v1 = work.tile([128, 24, 128], BF16, tag="v1")
            vw = work.tile([128, 24, 128], BF16, tag="vw")
            vwa = work.tile([128, 24, 64], BF16, tag="vwa")
            vwb = work.tile([128, 24, 32], BF16, tag="vwb")
            h2 = work.tile([128, 24], F32, tag="h2")
            h2c = work.tile([128, 24], F32, tag="h2c")
            h2s = work.tile([128, 24], F32, tag="h2s")
            h2d = work.tile([128, 24], F32, tag="h2d")
            e2_raw = work.tile([128, 144], F32, tag="e2raw")
            e2_lr = work.tile([128, 144], F32, tag="e2lr")
            E2 = work.tile([128, 144], F32, tag="E2")
            Z2 = work.tile([128, 24], F32, tag="Z2")
            rZ2 = work.tile([128, 24], F32, tag="rZ2")
            P2 = work.tile([128, 144], F32, tag="P2")
            S2 = work.tile([128, 24], F32, tag="S2")
            out_sb = work.tile([128, 24], F32, tag="outsb")

            def pe_pair(t0):
                # paired emission: same-stationary matmuls for both tiles
                # back to back (1 LDWEIGHTS per weight instead of per tile)
                hps = {}
                for t in (t0, t0 + 1):
                    g = gs[t // 2]
                    c0 = (t % 2) * 128
                    ts = slice(t * 128, (t + 1) * 128)
                    nc.tensor.matmul(shF[:, ts], w1bc_sb[:, 0, :],
                                     g[:, 6, c0:c0 + 128], start=False,
                                     stop=True)
                    nc.scalar.activation(sh_sb[:, t, :], shF[:, ts], ACT.Copy)
                for t in (t0, t0 + 1):
                    g = gs[t // 2]
                    c0 = (t % 2) * 128
                    h_ps = psH.tile([128, 768], F32, tag="hps")
                    hps[t] = h_ps
                    nc.tensor.matmul(h_ps[:, 0:512], w1bc_sb[:, 1, :],
                                     g[:, 0:4, c0:c0 + 128], start=True,
                                     stop=False)
                    nc.tensor.matmul(h_ps[:, 512:768], w1bc_sb[:, 1, :],
                                     g[:, 4:6, c0:c0 + 128], start=True,
                                     stop=False)
                for t in (t0, t0 + 1):
                    h_ps = hps[t]
                    for hs, nk in ((slice(0, 512), 4), (slice(512, 768), 2)):
                        mov = (sh_sb[:, t, :].unsqueeze(1)
                               .broadcast_to([128, nk, 128]))
                        nc.tensor.matmul(h_ps[:, hs], ident_sb[:, :], mov,
                                         start=False, stop=True)
                for t in (t0, t0 + 1):
                    pe_tri(t, hps[t])

            def pe_tri(t, h_ps):
                hT = htp.tile([128, 768], BF16, tag="hT")
                if t < 2:
                    # vector idles before the half-0 tail; scalar's serial
                    # eviction chain gates E1 -> do t0/t1's hT there
                    nc.vector.tensor_scalar(hT[:, :], h_ps[:, :],
                                            biash_sb[:, 0:1], None, OP.add)
                else:
                    nc.scalar.activation(hT[:, :], h_ps[:, :], ACT.Identity,
                                         bias=biash_sb[:, :], scale=1.0)
                # fused transpose + e via trident; 3+3 split keeps every
                # matmul output inside one PSUM bank.
                ps2a = psT.tile([128, 3, 136], F32, tag="ps2a")
                ps2b = psT.tile([128, 3, 136], F32, tag="ps2b")
                for k in range(K):
                    ps2k = ps2a[:, k, :] if k < 3 else ps2b[:, k - 3, :]
                    nc.tensor.matmul(
                        ps2k, hT[:, k * 128:(k + 1) * 128],
                        tri_sb[:, :], start=True, stop=True,
                    )
                # evictions: hbL casts on scalar, tiny e copies on vector
                nc.scalar.activation(
                    hbL[:, t, 0:384].rearrange("p (k f) -> p k f", k=3),
                    ps2a[:, :, 0:128], ACT.Copy)
                if t < 2:
                    nc.vector.tensor_copy(
                        hbL[:, t, 384:768].rearrange("p (k f) -> p k f", k=3),
                        ps2b[:, :, 0:128])
                else:
                    nc.scalar.activation(
                        hbL[:, t, 384:768].rearrange("p (k f) -> p k f", k=3),
                        ps2b[:, :, 0:128], ACT.Copy)
                nc.vector.tensor_copy(
                    e_sd[:, t, 0:24].rearrange("p (k s) -> p k s", k=3),
                    ps2a[:, :, 128:136])
                nc.vector.tensor_copy(
                    e_sd[:, t, 24:48].rearrange("p (k s) -> p k s", k=3),
                    ps2b[:, :, 128:136])
                if ce_nz:
                    nc.vector.tensor_tensor(
                        e_sd[:, t, :], e_sd[:, t, :], ce_sb[:, :], OP.subtract)

            def softmax_tile(t):
                # per-tile softmax chain: lets tile t's apply products
                # start as soon as ITS alpha is ready (latency-critical
                # for the first half; half-1 overlaps DVE work anyway)
                st = slice(t * 144, (t + 1) * 144)
                zt = slice(t * 24, (t + 1) * 24)
                nc.vector.tensor_tensor(
                    e_raw[:, t, :].rearrange("p (i j h) -> p i j h",
                                             j=6, h=4),
                    e4[:, t, :, 1, :].unsqueeze(2)
                    .broadcast_to([128, 6, 6, 4]),
                    e4[:, t, :, 0, :].unsqueeze(1)
                    .broadcast_to([128, 6, 6, 4]),
                    OP.add,
                )
                nc.vector.scalar_tensor_tensor(
                    e_lrf[:, st], e_rawf[:, st], LRELU_SLOPE, e_rawf[:, st],
                    OP.mult, OP.max)
                nc.scalar.activation(E1[:, st], e_lrf[:, st], ACT.Exp)
                nc.vector.tensor_reduce(
                    Z1[:, zt],
                    E1[:, st].rearrange("p (i j h) -> p i j h", j=6, h=4)
                    .transpose([0, 1, 3, 2]),
                    axis=AX.X, op=OP.add)
                nc.vector.reciprocal_approx_fast(rZ1[:, zt], Z1[:, zt])
                nc.vector.tensor_tensor(
                    al[:, t, :].rearrange("p (i j h) -> p i j h", j=6, h=4),
                    E1[:, st].rearrange("p (i j h) -> p i j h", j=6, h=4),
                    rZ1[:, zt].rearrange("p (i h) -> p i h", h=4)
                    .unsqueeze(2).broadcast_to([128, 6, 6, 4]),
                    OP.mult,
                )
                for i in range(K):
                    nc.vector.tensor_tensor(
                        prod[:, t * 6 + i, :].rearrange(
                            "p (j f h) -> p j f h", f=32, h=4),
                        al[:, t, i * 24:(i + 1) * 24]
                        .rearrange("p (j h) -> p j h", h=4)
                        .unsqueeze(2).broadcast_to([128, 6, 32, 4]),
                        hbL[:, t, :].rearrange("p (j f h) -> p j f h",
                                               f=32, h=4),
                        OP.mult,
                    )

            def tail_half(half):
                th0 = 2 * half
                cs = slice(th0 * 6, (th0 + 2) * 6)
                softmax_tile(th0)
                softmax_tile(th0 + 1)
                nc.vector.tensor_add(f1[:, cs, :], prod[:, cs, 0:384],
                                     prod[:, cs, 384:768])
                nc.vector.tensor_add(f2[:, cs, :], f1[:, cs, 0:128],
                                     f1[:, cs, 128:256])
                nc.vector.tensor_add(attn[:, cs, :], f2[:, cs, :],
                                     f1[:, cs, 256:384])
                # elu(x)+1 = exp(min(x,0)) + relu(x)  (-1 folded into c2);
                # min/relu at 4x tensor_scalar rate, exp on scalar.
                nc.vector.tensor_scalar_min(minat[:, cs, :], attn[:, cs, :],
                                            0.0)
                nc.vector.tensor_scalar_max(relux[:, cs, :], attn[:, cs, :],
                                            0.0)

            def tail_b(half):
                cs = slice(2 * half * 6, (2 * half + 2) * 6)
                for t in (2 * half, 2 * half + 1):
                    c6 = slice(t * 6, (t + 1) * 6)
                    nc.scalar.activation(expm[:, c6, :], minat[:, c6, :],
                                         ACT.Exp)
                    nc.vector.tensor_add(v1[:, c6, :], expm[:, c6, :],
                                         relux[:, c6, :])
                nc.vector.tensor_tensor(
                    vw[:, cs, :], v1[:, cs, :],
                    w2_sb[:, :].unsqueeze(1).broadcast_to([128, 12, 128]),
                    OP.mult)
                # fold W2-dot with cheap 2x adds, then a short 1x reduce
                nc.vector.tensor_add(vwa[:, cs, :], vw[:, cs, 0:64],
                                     vw[:, cs, 64:128])
                nc.vector.tensor_add(vwb[:, cs, :], vwa[:, cs, 0:32],
                                     vwa[:, cs, 32:64])
                nc.vector.tensor_reduce(h2[:, cs], vwb[:, cs, :], axis=AX.X,
                                        op=OP.add)

            def l2_half(half):
                # layer-2 GAT attention over the 6 nodes, per tile pair
                cs = slice(half * 12, (half + 1) * 12)
                nc.vector.tensor_scalar(h2c[:, cs], h2[:, cs], -c2, None,
                                        OP.add)
                nc.vector.tensor_scalar(h2s[:, cs], h2[:, cs], as2,
                                        -c2 * (as2 + ad2), OP.mult, OP.add)
                nc.vector.tensor_scalar(h2d[:, cs], h2[:, cs], ad2, None,
                                        OP.mult)
                h2dv = h2d[:, cs].rearrange("p (t i) -> p t i", t=2)
                h2sv = h2s[:, cs].rearrange("p (t j) -> p t j", t=2)
                h2cv = h2c[:, cs].rearrange("p (t j) -> p t j", t=2)
                es = slice(half * 72, (half + 1) * 72)
                nc.vector.tensor_tensor(
                    e2_raw[:, es].rearrange("p (t i j) -> p t i j", t=2, j=6),
                    h2dv.unsqueeze(3).broadcast_to([128, 2, 6, 6]),
                    h2sv.unsqueeze(2).broadcast_to([128, 2, 6, 6]),
                    OP.add,
                )
                nc.vector.scalar_tensor_tensor(
                    e2_lr[:, es], e2_raw[:, es], LRELU_SLOPE, e2_raw[:, es],
                    OP.mult, OP.max)
                nc.scalar.activation(E2[:, es], e2_lr[:, es], ACT.Exp)
                nc.vector.tensor_reduce(
                    Z2[:, cs],
                    E2[:, es].rearrange("p (ti j) -> p ti j", j=6),
                    axis=AX.X, op=OP.add)
                nc.vector.reciprocal_approx_fast(rZ2[:, cs], Z2[:, cs])
                nc.vector.tensor_tensor(
                    P2[:, es].rearrange("p (t i j) -> p t i j", t=2, j=6),
                    E2[:, es].rearrange("p (t i j) -> p t i j", t=2, j=6),
                    h2cv.unsqueeze(2).broadcast_to([128, 2, 6, 6]),
                    OP.mult,
                )
                nc.vector.tensor_reduce(
                    S2[:, cs],
                    P2[:, es].rearrange("p (ti j) -> p ti j", j=6),
                    axis=AX.X, op=OP.add,
                )
                nc.vector.tensor_mul(out_sb[:, cs], S2[:, cs], rZ2[:, cs])
                if b2 != 0.0:
                    nc.vector.tensor_scalar(out_sb[:, cs], out_sb[:, cs], b2,
                                            None, OP.add)
                nc.sync.dma_start(
                    out=bass.AP(tensor=out_ext, offset=half * 2 * 128 * K,
                                ap=[[K, 128], [128 * K, 2], [1, K]]),
                    in_=out_sb[:, cs].rearrange("p (t k) -> p t k", t=2))

            # emission order = per-engine program order: keep each half's
            # tail right after its two tiles so no engine's queue blocks
            # half-0 work behind tile-2/3 dependencies.
            pe_pair(0)
            tail_half(0)
            pe_pair(2)
            tail_b(0)
            l2_half(0)
            tail_half(1)
            tail_b(1)
            l2_half(1)

    nc.finalize()
    return nc


def prep_shared(inp):
    """Host-side layout prep shared across cores (index math / weight
    folding / dtype casts only -- all tensor FLOPs on the batch stay on
    device)."""
    bf = ml_dtypes.bfloat16
    node = np.asarray(inp["node_embeddings"], np.float32).astype(bf)
    sub = np.asarray(inp["substation_embeddings"], np.float32).astype(bf)
    elem = np.asarray(inp["sub_id_to_elem_id"], np.int64)
    tbl = np.empty((B, S, RB, H), bf)
    tbl[:, :, 0:K, :] = node[:, elem.reshape(-1), :].reshape(B, S, K, H)
    tbl[:, :, 6, :] = sub
    sub_idx = np.asarray(inp["sub_choice"], np.int64).reshape(B)
    obs = np.asarray(inp["org_obs"], np.float32)

    W1 = np.asarray(inp["W1"], np.float32)
    W1a, W1b, W1c = W1[0:H], W1[H:2 * H], W1[2 * H:3 * H]
    Wp = np.asarray(inp["W_proj"], np.float32)
    wfold = np.zeros((OBS_PAD, H), np.float32)
    wfold[:OBS] = Wp @ W1a
    wfold4 = wfold.reshape(4, 128, H).transpose(1, 0, 2).copy()
    bias_h = (np.asarray(inp["b1"], np.float32)
              + np.asarray(inp["b_proj"], np.float32) @ W1a)

    a_src1 = np.asarray(inp["a_src1"], np.float32)
    a_dst1 = np.asarray(inp["a_dst1"], np.float32)
    asrc_m = np.zeros((H, 8), np.float32)
    for h in range(HEADS):
        asrc_m[h * FH:(h + 1) * FH, h] = a_src1[h]
        asrc_m[h * FH:(h + 1) * FH, 4 + h] = a_dst1[h]
    # first 128 cols: permutation (h,f)->(f,h) so the transpose emits the
    # apply-friendly layout directly
    perm = np.zeros((H, H), np.float32)
    for h in range(HEADS):
        for f in range(FH):
            perm[h * FH + f, f * HEADS + h] = 1.0
    trident = np.concatenate([perm, asrc_m], axis=1)

    # e must be computed from h WITHOUT b1 (but WITH the obs-projection
    # bias, which is part of the reference h) -> correct only for b1.
    bh = np.asarray(inp["b1"], np.float32).reshape(HEADS, FH)
    cek = np.zeros((K, 8), np.float32)
    cek[:, 0:4] = (bh * a_src1).sum(-1)[None, :]
    cek[:, 4:8] = (bh * a_dst1).sum(-1)[None, :]

    shared = {
        "tbl": tbl,
        "sub_idx": sub_idx,
        "obs": obs,
        "wfold": wfold4.astype(bf),
        "w1bc": np.stack([W1b, W1c], axis=1).astype(bf),
        "trident": trident.astype(bf),
        "biash": bias_h.reshape(H, 1).astype(np.float32),
        "ce": cek.reshape(1, 48).astype(np.float32),
        # W2 permuted to the (f,h) feature order used by the apply layout
        "w2r": np.asarray(inp["W2"], np.float32).reshape(HEADS, FH).T
               .reshape(1, H).astype(bf),
        "identd": np.eye(H, dtype=np.float32).astype(bf),
    }
    return shared


def prep_core_inputs(core, shared):
    bf = ml_dtypes.bfloat16
    s = slice(core * BS, (core + 1) * BS)

    obs_T = np.zeros((OBS_PAD, BS), np.float32)
    obs_T[:OBS, :] = shared["obs"][s].T
    obs_T4 = obs_T.reshape(4, 128, BS).transpose(1, 0, 2).copy()

    sub_idx = shared["sub_idx"][s]
    idx_cols = []
    for p in range(2):
        bl = np.arange(256) + p * 256
        v = (bl * S + sub_idx[bl]).astype(np.int16)
        blk = v.reshape(16, 16).T
        idx_cols.append(np.tile(blk, (8, 1)))
    idx2 = np.ascontiguousarray(np.concatenate(idx_cols, axis=1))

    return {
        "table7": shared["tbl"][s].reshape(BS * S, RB * H),
        "idx2": idx2,
        "obs_T": obs_T4.astype(bf),
        "wfold": shared["wfold"],
        "w1bc": shared["w1bc"],
        "trident": shared["trident"],
        "biash": shared["biash"],
        "ce": shared["ce"],
        "w2r": shared["w2r"],
        "identd": shared["identd"],
    }


_GRAPH_CACHE = {}
LAST_RESULTS = None


def kernel(**inputs):
    inp = {k: np.asarray(v) for k, v in inputs.items()}
    W2 = np.asarray(inp["W2"], np.float32)
    shared = prep_shared(inp)
    scalars = {
        "a_src2": float(np.asarray(inp["a_src2"]).reshape(-1)[0]),
        "a_dst2": float(np.asarray(inp["a_dst2"]).reshape(-1)[0]),
        "b2": float(np.asarray(inp["b2"]).reshape(-1)[0]),
        "c2": float(W2.sum()),
        "ce_nz": bool(np.any(np.abs(shared["ce"]) > 0)),
    }
    key = tuple(sorted(scalars.items()))
    if key not in _GRAPH_CACHE:
        _GRAPH_CACHE[key] = build_graph(scalars)
    nc = _GRAPH_CACHE[key]

    in_maps = [prep_core_inputs(c, shared) for c in range(NCORES)]
    res = run_bass_kernel_spmd(nc, in_maps, core_ids=list(range(NCORES)))
    global LAST_RESULTS
    LAST_RESULTS = res
    out = np.concatenate([res.results[c]["out"] for c in range(NCORES)], axis=0)
    return out.reshape(B, K, 1).astype(np.float32)


if __name__ == "__main__":
    g = build_graph({"a_src2": 0.01, "a_dst2": 0.02, "b2": 0.0, "c2": 0.1,
                     "ce_nz": False})
    print("graph built ok")


# revision 33
# speedup vs baseline: 1.0130x; 1.0130x over previous
"""Trainium2 Bass kernel for nn_Action_Decoder (GAT-based action decoder).

v3 strategy (8 NeuronCores, pure data-parallel over batch):
  - B=4096 sharded 8 x 512 samples/core; weights replicated; 4 tiles of
    128 samples on the partition dim.
  - Gather restructure: host packs a [BS*36, 7*128] bf16 block table
    (rows for (sample, substation) = 6 node rows + 1 sub row, built with
    sample-INDEPENDENT indexing from the fixed [36,6] element table).
    The device gathers ONE 1792-byte block per sample: 2 dma_gather ops
    of 256 idxs each (vs 8 ops / 3584 descriptors in v2) -> SWDGE
    descriptor generation drops ~7x and blocks land directly in x^T
    layout ([feature, k, sample]).
  - idx DMA issued first on the sync queue; consts spread across
    vector/tensor/scalar queues so nothing delays the gather.
  - Layer-1 on PE as v2 (obs folded via W_proj@W1a, shared obs+sub part
    broadcast over the 6 nodes with an identity-stationary matmul,
    fused transpose+e via the trident matmul).
  - Tail engine rebalance: ALL PSUM evictions + exps on Scalar, e_raw /
    Z1 / relu / one alpha*h product row on GpSimd, DVE keeps the
    irreducible tensor-tensor chain (apply + folds + elu-combine + W2).
  - elu fused: elu(x)+1 = min(exp(x),1) + relu(x), one scalar Exp + one
    DVE scalar_tensor_tensor; W2 dot via one mult + one axis-reduce.
"""

import os
import sys

import numpy as np

for _p in ("/root/.axon_site", "/root/.axon_site/_ro/trn_rl_repo",
           "/root/.axon_site/_ro/pypackages", "/opt/trn_rl_repo", "/opt/pypackages"):
    if os.path.isdir(_p) and _p not in sys.path:
        sys.path.append(_p)

import ml_dtypes

import concourse.bass as bass
import concourse.tile as tile
from concourse import bacc
from concourse import mybir
from concourse.bass_utils import run_bass_kernel_spmd

# Problem dims
B, N, S, K, H, OBS = 4096, 177, 36, 6, 128, 500
HEADS, FH = 4, 32
NCORES = 8
BS = B // NCORES          # 512 samples per core
NT = BS // 128            # 4 tiles of 128 samples
OBS_PAD = 512             # pad 500 -> 512
RB = 7                    # rows per (sample, substation) block

F32 = mybir.dt.float32
BF16 = mybir.dt.bfloat16
I16 = mybir.dt.int16
AX = mybir.AxisListType
OP = mybir.AluOpType
ACT = mybir.ActivationFunctionType

LRELU_SLOPE = 0.2
# GpSimd does SWDGE gathers ONLY. Measured: any gpsimd elementwise op
# running concurrently with DVE slows BOTH ~3-40x (shared SBUF port),
# and using gpsimd tensor ops forces a Pool ucode lib swap (~12us).


def build_graph(scalars):
    as2 = float(scalars["a_src2"])
    ad2 = float(scalars["a_dst2"])
    b2 = float(scalars["b2"])
    c2 = float(scalars["c2"])
    ce_nz = bool(scalars["ce_nz"])

    nc = bacc.Bacc(num_swdge_queues=2)

    table7 = nc.declare_dram_parameter("table7", [BS * S, RB * H], BF16,
                                       isOutput=False)
    idx2 = nc.declare_dram_parameter("idx2", [128, 32], I16, isOutput=False)
    obs_T = nc.declare_dram_parameter("obs_T", [128, 4, BS], BF16, isOutput=False)
    wfold = nc.declare_dram_parameter("wfold", [128, 4, H], BF16, isOutput=False)
    w1bc = nc.declare_dram_parameter("w1bc", [H, 2, H], BF16, isOutput=False)
    trident = nc.declare_dram_parameter("trident", [H, 136], BF16, isOutput=False)
    identd = nc.declare_dram_parameter("identd", [H, H], BF16, isOutput=False)
    biash = nc.declare_dram_parameter("biash", [H, 1], F32, isOutput=False)
    ce = nc.declare_dram_parameter("ce", [1, 48], F32, isOutput=False)
    w2r = nc.declare_dram_parameter("w2r", [1, H], BF16, isOutput=False)
    out_ext = nc.declare_dram_parameter("out", [BS, K], F32, isOutput=True)

    with tile.TileContext(nc) as tc:
        with (
            tc.tile_pool(name="consts", bufs=1) as consts,
            tc.tile_pool(name="gat", bufs=1) as gat,
            tc.tile_pool(name="ht", bufs=2) as htp,
            tc.tile_pool(name="work", bufs=1) as work,
            tc.tile_pool(name="psS", bufs=1, space="PSUM") as psS,
            tc.tile_pool(name="psH", bufs=2, space="PSUM") as psH,
            tc.tile_pool(name="psT", bufs=1, space="PSUM") as psT,
        ):
            # ---- idx first (the gathers' only dependency), on sync ----
            idx_sb = consts.tile([128, 32], I16)
            nc.sync.dma_start(out=idx_sb, in_=idx2[:, :])

            # ---- consts spread over the sync + scalar queues ----
            obs_sb = consts.tile([128, 4, BS], BF16)
            nc.sync.dma_start(out=obs_sb[:, 0:2, :], in_=obs_T[:, 0:2, :])
            nc.scalar.dma_start(out=obs_sb[:, 2:4, :], in_=obs_T[:, 2:4, :])
            wfold_sb = consts.tile([128, 4, 128], BF16)
            nc.scalar.dma_start(out=wfold_sb[:, :, :], in_=wfold[:, :, :])
            w1bc_sb = consts.tile([128, 2, 128], BF16)
            nc.sync.dma_start(out=w1bc_sb[:, :, :], in_=w1bc[:, :, :])
            tri_sb = consts.tile([128, 136], BF16)
            nc.sync.dma_start(out=tri_sb, in_=trident[:, :])
            ident_sb = consts.tile([128, 128], BF16)
            nc.sync.dma_start(out=ident_sb, in_=identd[:, :])
            biash_sb = consts.tile([128, 1], F32)
            nc.scalar.dma_start(out=biash_sb, in_=biash[:, :])
            w2_sb = consts.tile([128, 128], BF16)
            nc.scalar.dma_start(
                out=w2_sb,
                in_=bass.AP(tensor=w2r, offset=0, ap=[[0, 128], [1, 128]]),
            )
            if ce_nz:
                ce_sb = consts.tile([128, 48], F32)
                nc.scalar.dma_start(
                    out=ce_sb,
                    in_=bass.AP(tensor=ce, offset=0, ap=[[0, 128], [1, 48]]),
                )

            # ---- gathers: one 1792B block per sample, 256 idx per pair.
            # The first SWDGE instruction absorbs the Pool ucode lib-load
            # latency (~11.5us after MODIFY_POOL_CONFIG). ----
            gs = []
            for p in range(2):
                g = gat.tile([128, RB, 256], BF16, tag=f"g{p}")
                nc.gpsimd.dma_gather(
                    out_ap=g[:, :, :],
                    in_ap=table7[:, :],
                    idxs_ap=idx_sb[:, p * 16:(p + 1) * 16],
                    num_idxs=256, num_idxs_reg=256, elem_size=RB * H,
                    transpose=True, queue_num=p,
                )
                gs.append(g)

            # ---- shared part: obs (4 accumulating chunks over all 512
            # samples) + per-tile sub row. First psS write must be the only
            # start=True on that bank. ----
            shF = psS.tile([128, 512], F32)
            for c in range(4):
                nc.tensor.matmul(shF[:, :], wfold_sb[:, c, :], obs_sb[:, c, :],
                                 start=(c == 0), stop=False)

            sh_sb = work.tile([128, 4, 128], BF16, tag="sh")
            hbL = work.tile([128, NT, 768], BF16, tag="hbL")
            e_sd = work.tile([128, NT, 48], F32, tag="esd")
            e4 = e_sd[:, :, :].rearrange("p t (k s h) -> p t k s h", s=2, h=4)
            e_raw = work.tile([128, NT, 144], BF16, tag="eraw")
            e_rawf = e_raw[:, :, :].rearrange("p t x -> p (t x)")
            e_lrf = work.tile([128, NT * 144], BF16, tag="elr")
            E1 = work.tile([128, NT * 144], F32, tag="E1")
            Z1 = work.tile([128, 96], F32, tag="Z1")
            rZ1 = work.tile([128, 96], F32, tag="rZ1")
            al = work.tile([128, NT, 144], BF16, tag="al")
            prod = work.tile([128, 24, 768], BF16, tag="prod")
            f1 = work.tile([128, 24, 384], BF16, tag="f1")
            f2 = work.tile([128, 24, 128], BF16, tag="f2")
            attn = work.tile([128, 24, 128], BF16, tag="attn")
            minat = work.tile([128, 24, 128], BF16, tag="minat")
            expm = work.tile([128, 24, 128], BF16, tag="expm")
            relux = work.tile([128, 24, 128], BF16, tag="relux")
            v1 = work.tile([128, 24, 128], BF16, tag="v1")
            vw = work.tile([128, 24, 128], BF16, tag="vw")
            vwa = work.tile([128, 24, 64], BF16, tag="vwa")
            vwb = work.tile([128, 24, 32], BF16, tag="vwb")
            h2 = work.tile([128, 24], F32, tag="h2")
            h2c = work.tile([128, 24], F32, tag="h2c")
            h2s = work.tile([128, 24], F32, tag="h2s")
            h2d = work.tile([128, 24], F32, tag="h2d")
            e2_raw = work.tile([128, 144], F32, tag="e2raw")
            e2_lr = work.tile([128, 144], F32, tag="e2lr")
            E2 = work.tile([128, 144], F32, tag="E2")
            Z2 = work.tile([128, 24], F32, tag="Z2")
            rZ2 = work.tile([128, 24], F32, tag="rZ2")
            P2 = work.tile([128, 144], F32, tag="P2")
            S2 = work.tile([128, 24], F32, tag="S2")
            out_sb = work.tile([128, 24], F32, tag="outsb")

            def pe_pair(t0):
                # paired emission: same-stationary matmuls for both tiles
                # back to back (1 LDWEIGHTS per weight instead of per tile)
                hps = {}
                for t in (t0, t0 + 1):
                    g = gs[t // 2]
                    c0 = (t % 2) * 128
                    ts = slice(t * 128, (t + 1) * 128)
                    nc.tensor.matmul(shF[:, ts], w1bc_sb[:, 0, :],
                                     g[:, 6, c0:c0 + 128], start=False,
                                     stop=True)
                    nc.scalar.activation(sh_sb[:, t, :], shF[:, ts], ACT.Copy)
                for t in (t0, t0 + 1):
                    g = gs[t // 2]
                    c0 = (t % 2) * 128
                    h_ps = psH.tile([128, 768], F32, tag="hps")
                    hps[t] = h_ps
                    nc.tensor.matmul(h_ps[:, 0:512], w1bc_sb[:, 1, :],
                                     g[:, 0:4, c0:c0 + 128], start=True,
                                     stop=False)
                    nc.tensor.matmul(h_ps[:, 512:768], w1bc_sb[:, 1, :],
                                     g[:, 4:6, c0:c0 + 128], start=True,
                                     stop=False)
                for t in (t0, t0 + 1):
                    h_ps = hps[t]
                    for hs, nk in ((slice(0, 512), 4), (slice(512, 768), 2)):
                        mov = (sh_sb[:, t, :].unsqueeze(1)
                               .broadcast_to([128, nk, 128]))
                        nc.tensor.matmul(h_ps[:, hs], ident_sb[:, :], mov,
                                         start=False, stop=True)
                for t in (t0, t0 + 1):
                    pe_tri(t, hps[t])

            def pe_tri(t, h_ps):
                hT = htp.tile([128, 768], BF16, tag="hT")
                if t < 2:
                    # vector idles before the half-0 tail; scalar's serial
                    # eviction chain gates E1 -> do t0/t1's hT there
                    nc.vector.tensor_scalar(hT[:, :], h_ps[:, :],
                                            biash_sb[:, 0:1], None, OP.add)
                else:
                    nc.scalar.activation(hT[:, :], h_ps[:, :], ACT.Identity,
                                         bias=biash_sb[:, :], scale=1.0)
                # fused transpose + e via trident; 3+3 split keeps every
                # matmul output inside one PSUM bank.
                ps2a = psT.tile([128, 3, 136], F32, tag="ps2a")
                ps2b = psT.tile([128, 3, 136], F32, tag="ps2b")
                for k in range(K):
                    ps2k = ps2a[:, k, :] if k < 3 else ps2b[:, k - 3, :]
                    nc.tensor.matmul(
                        ps2k, hT[:, k * 128:(k + 1) * 128],
                        tri_sb[:, :], start=True, stop=True,
                    )
                # evictions: hbL casts on scalar, tiny e copies on vector
                nc.scalar.activation(
                    hbL[:, t, 0:384].rearrange("p (k f) -> p k f", k=3),
                    ps2a[:, :, 0:128], ACT.Copy)
                if t < 2:
                    nc.vector.tensor_copy(
                        hbL[:, t, 384:768].rearrange("p (k f) -> p k f", k=3),
                        ps2b[:, :, 0:128])
                else:
                    nc.scalar.activation(
                        hbL[:, t, 384:768].rearrange("p (k f) -> p k f", k=3),
                        ps2b[:, :, 0:128], ACT.Copy)
                nc.vector.tensor_copy(
                    e_sd[:, t, 0:24].rearrange("p (k s) -> p k s", k=3),
                    ps2a[:, :, 128:136])
                nc.vector.tensor_copy(
                    e_sd[:, t, 24:48].rearrange("p (k s) -> p k s", k=3),
                    ps2b[:, :, 128:136])
                if ce_nz:
                    nc.vector.tensor_tensor(
                        e_sd[:, t, :], e_sd[:, t, :], ce_sb[:, :], OP.subtract)

            def softmax_tile(t):
                # per-tile softmax chain: lets tile t's apply products
                # start as soon as ITS alpha is ready (latency-critical
                # for the first half; half-1 overlaps DVE work anyway)
                st = slice(t * 144, (t + 1) * 144)
                zt = slice(t * 24, (t + 1) * 24)
                nc.vector.tensor_tensor(
                    e_raw[:, t, :].rearrange("p (i j h) -> p i j h",
                                             j=6, h=4),
                    e4[:, t, :, 1, :].unsqueeze(2)
                    .broadcast_to([128, 6, 6, 4]),
                    e4[:, t, :, 0, :].unsqueeze(1)
                    .broadcast_to([128, 6, 6, 4]),
                    OP.add,
                )
                nc.vector.scalar_tensor_tensor(
                    e_lrf[:, st], e_rawf[:, st], LRELU_SLOPE, e_rawf[:, st],
                    OP.mult, OP.max)
                nc.scalar.activation(E1[:, st], e_lrf[:, st], ACT.Exp)
                nc.vector.tensor_reduce(
                    Z1[:, zt],
                    E1[:, st].rearrange("p (i j h) -> p i j h", j=6, h=4)
                    .transpose([0, 1, 3, 2]),
                    axis=AX.X, op=OP.add)
                nc.vector.reciprocal_approx_fast(rZ1[:, zt], Z1[:, zt])
                nc.vector.tensor_tensor(
                    al[:, t, :].rearrange("p (i j h) -> p i j h", j=6, h=4),
                    E1[:, st].rearrange("p (i j h) -> p i j h", j=6, h=4),
                    rZ1[:, zt].rearrange("p (i h) -> p i h", h=4)
                    .unsqueeze(2).broadcast_to([128, 6, 6, 4]),
                    OP.mult,
                )
                for i in range(K):
                    nc.vector.tensor_tensor(
                        prod[:, t * 6 + i, :].rearrange(
                            "p (j f h) -> p j f h", f=32, h=4),
                        al[:, t, i * 24:(i + 1) * 24]
                        .rearrange("p (j h) -> p j h", h=4)
                        .unsqueeze(2).broadcast_to([128, 6, 32, 4]),
                        hbL[:, t, :].rearrange("p (j f h) -> p j f h",
                                               f=32, h=4),
                        OP.mult,
                    )

            def tail_half(half):
                th0 = 2 * half
                cs = slice(th0 * 6, (th0 + 2) * 6)
                softmax_tile(th0)
                softmax_tile(th0 + 1)
                nc.vector.tensor_add(f1[:, cs, :], prod[:, cs, 0:384],
                                     prod[:, cs, 384:768])
                nc.vector.tensor_add(f2[:, cs, :], f1[:, cs, 0:128],
                                     f1[:, cs, 128:256])
                nc.vector.tensor_add(attn[:, cs, :], f2[:, cs, :],
                                     f1[:, cs, 256:384])
                # elu(x)+1 = exp(min(x,0)) + relu(x)  (-1 folded into c2);
                # min/relu at 4x tensor_scalar rate, exp on scalar.
                nc.vector.tensor_scalar_min(minat[:, cs, :], attn[:, cs, :],
                                            0.0)
                nc.vector.tensor_scalar_max(relux[:, cs, :], attn[:, cs, :],
                                            0.0)

            def tail_b(half):
                cs = slice(2 * half * 6, (2 * half + 2) * 6)
                for t in (2 * half, 2 * half + 1):
                    c6 = slice(t * 6, (t + 1) * 6)
                    nc.scalar.activation(expm[:, c6, :], minat[:, c6, :],
                                         ACT.Exp)
                    nc.vector.tensor_add(v1[:, c6, :], expm[:, c6, :],
                                         relux[:, c6, :])
                nc.vector.tensor_tensor(
                    vw[:, cs, :], v1[:, cs, :],
                    w2_sb[:, :].unsqueeze(1).broadcast_to([128, 12, 128]),
                    OP.mult)
                # fold W2-dot with cheap 2x adds, then a short 1x reduce
                nc.vector.tensor_add(vwa[:, cs, :], vw[:, cs, 0:64],
                                     vw[:, cs, 64:128])
                nc.vector.tensor_add(vwb[:, cs, :], vwa[:, cs, 0:32],
                                     vwa[:, cs, 32:64])
                nc.vector.tensor_reduce(h2[:, cs], vwb[:, cs, :], axis=AX.X,
                                        op=OP.add)

            def l2_half(half):
                # layer-2 GAT attention over the 6 nodes, per tile pair
                cs = slice(half * 12, (half + 1) * 12)
                nc.vector.tensor_scalar(h2c[:, cs], h2[:, cs], -c2, None,
                                        OP.add)
                nc.vector.tensor_scalar(h2s[:, cs], h2[:, cs], as2,
                                        -c2 * (as2 + ad2), OP.mult, OP.add)
                nc.vector.tensor_scalar(h2d[:, cs], h2[:, cs], ad2, None,
                                        OP.mult)
                h2dv = h2d[:, cs].rearrange("p (t i) -> p t i", t=2)
                h2sv = h2s[:, cs].rearrange("p (t j) -> p t j", t=2)
                h2cv = h2c[:, cs].rearrange("p (t j) -> p t j", t=2)
                es = slice(half * 72, (half + 1) * 72)
                nc.vector.tensor_tensor(
                    e2_raw[:, es].rearrange("p (t i j) -> p t i j", t=2, j=6),
                    h2dv.unsqueeze(3).broadcast_to([128, 2, 6, 6]),
                    h2sv.unsqueeze(2).broadcast_to([128, 2, 6, 6]),
                    OP.add,
                )
                nc.vector.scalar_tensor_tensor(
                    e2_lr[:, es], e2_raw[:, es], LRELU_SLOPE, e2_raw[:, es],
                    OP.mult, OP.max)
                nc.scalar.activation(E2[:, es], e2_lr[:, es], ACT.Exp)
                nc.vector.tensor_reduce(
                    Z2[:, cs],
                    E2[:, es].rearrange("p (ti j) -> p ti j", j=6),
                    axis=AX.X, op=OP.add)
                nc.vector.reciprocal_approx_fast(rZ2[:, cs], Z2[:, cs])
                nc.vector.tensor_tensor(
                    P2[:, es].rearrange("p (t i j) -> p t i j", t=2, j=6),
                    E2[:, es].rearrange("p (t i j) -> p t i j", t=2, j=6),
                    h2cv.unsqueeze(2).broadcast_to([128, 2, 6, 6]),
                    OP.mult,
                )
                nc.vector.tensor_reduce(
                    S2[:, cs],
                    P2[:, es].rearrange("p (ti j) -> p ti j", j=6),
                    axis=AX.X, op=OP.add,
                )
                nc.vector.tensor_mul(out_sb[:, cs], S2[:, cs], rZ2[:, cs])
                if b2 != 0.0:
                    nc.vector.tensor_scalar(out_sb[:, cs], out_sb[:, cs], b2,
                                            None, OP.add)
                nc.sync.dma_start(
                    out=bass.AP(tensor=out_ext, offset=half * 2 * 128 * K,
                                ap=[[K, 128], [128 * K, 2], [1, K]]),
                    in_=out_sb[:, cs].rearrange("p (t k) -> p t k", t=2))

            # emission order = per-engine program order: keep each half's
            # tail right after its two tiles so no engine's queue blocks
            # half-0 work behind tile-2/3 dependencies.
            pe_pair(0)
            tail_half(0)
            pe_pair(2)
            tail_b(0)
            l2_half(0)
            tail_half(1)
            tail_b(1)
            l2_half(1)

    nc.finalize()
    return nc


def prep_shared(inp):
    """Host-side layout prep shared across cores (index math / weight
    folding / dtype casts only -- all tensor FLOPs on the batch stay on
    device)."""
    bf = ml_dtypes.bfloat16
    node = np.asarray(inp["node_embeddings"], np.float32).astype(bf)
    sub = np.asarray(inp["substation_embeddings"], np.float32).astype(bf)
    elem = np.asarray(inp["sub_id_to_elem_id"], np.int64)
    tbl = np.empty((B, S, RB, H), bf)
    tbl[:, :, 0:K, :] = node[:, elem.reshape(-1), :].reshape(B, S, K, H)
    tbl[:, :, 6, :] = sub
    sub_idx = np.asarray(inp["sub_choice"], np.int64).reshape(B)
    obs = np.asarray(inp["org_obs"], np.float32)

    W1 = np.asarray(inp["W1"], np.float32)
    W1a, W1b, W1c = W1[0:H], W1[H:2 * H], W1[2 * H:3 * H]
    Wp = np.asarray(inp["W_proj"], np.float32)
    wfold = np.zeros((OBS_PAD, H), np.float32)
    wfold[:OBS] = Wp @ W1a
    wfold4 = wfold.reshape(4, 128, H).transpose(1, 0, 2).copy()
    bias_h = (np.asarray(inp["b1"], np.float32)
              + np.asarray(inp["b_proj"], np.float32) @ W1a)

    a_src1 = np.asarray(inp["a_src1"], np.float32)
    a_dst1 = np.asarray(inp["a_dst1"], np.float32)
    asrc_m = np.zeros((H, 8), np.float32)
    for h in range(HEADS):
        asrc_m[h * FH:(h + 1) * FH, h] = a_src1[h]
        asrc_m[h * FH:(h + 1) * FH, 4 + h] = a_dst1[h]
    # first 128 cols: permutation (h,f)->(f,h) so the transpose emits the
    # apply-friendly layout directly
    perm = np.zeros((H, H), np.float32)
    for h in range(HEADS):
        for f in range(FH):
            perm[h * FH + f, f * HEADS + h] = 1.0
    trident = np.concatenate([perm, asrc_m], axis=1)

    # e must be computed from h WITHOUT b1 (but WITH the obs-projection
    # bias, which is part of the reference h) -> correct only for b1.
    bh = np.asarray(inp["b1"], np.float32).reshape(HEADS, FH)
    cek = np.zeros((K, 8), np.float32)
    cek[:, 0:4] = (bh * a_src1).sum(-1)[None, :]
    cek[:, 4:8] = (bh * a_dst1).sum(-1)[None, :]

    shared = {
        "tbl": tbl,
        "sub_idx": sub_idx,
        "obs": obs,
        "wfold": wfold4.astype(bf),
        "w1bc": np.stack([W1b, W1c], axis=1).astype(bf),
        "trident": trident.astype(bf),
        "biash": bias_h.reshape(H, 1).astype(np.float32),
        "ce": cek.reshape(1, 48).astype(np.float32),
        # W2 permuted to the (f,h) feature order used by the apply layout
        "w2r": np.asarray(inp["W2"], np.float32).reshape(HEADS, FH).T
               .reshape(1, H).astype(bf),
        "identd": np.eye(H, dtype=np.float32).astype(bf),
    }
    return shared


def prep_core_inputs(core, shared):
    bf = ml_dtypes.bfloat16
    s = slice(core * BS, (core + 1) * BS)

    obs_T = np.zeros((OBS_PAD, BS), np.float32)
    obs_T[:OBS, :] = shared["obs"][s].T
    obs_T4 = obs_T.reshape(4, 128, BS).transpose(1, 0, 2).copy()

    sub_idx = shared["sub_idx"][s]
    idx_cols = []
    for p in range(2):
        bl = np.arange(256) + p * 256
        v = (bl * S + sub_idx[bl]).astype(np.int16)
        blk = v.reshape(16, 16).T
        idx_cols.append(np.tile(blk, (8, 1)))
    idx2 = np.ascontiguousarray(np.concatenate(idx_cols, axis=1))

    return {
        "table7": shared["tbl"][s].reshape(BS * S, RB * H),
        "idx2": idx2,
        "obs_T": obs_T4.astype(bf),
        "wfold": shared["wfold"],
        "w1bc": shared["w1bc"],
        "trident": shared["trident"],
        "biash": shared["biash"],
        "ce": shared["ce"],
        "w2r": shared["w2r"],
        "identd": shared["identd"],
    }


_GRAPH_CACHE = {}
LAST_RESULTS = None


def kernel(**inputs):
    inp = {k: np.asarray(v) for k, v in inputs.items()}
    W2 = np.asarray(inp["W2"], np.float32)
    shared = prep_shared(inp)
    scalars = {
        "a_src2": float(np.asarray(inp["a_src2"]).reshape(-1)[0]),
        "a_dst2": float(np.asarray(inp["a_dst2"]).reshape(-1)[0]),
        "b2": float(np.asarray(inp["b2"]).reshape(-1)[0]),
        "c2": float(W2.sum()),
        "ce_nz": bool(np.any(np.abs(shared["ce"]) > 0)),
    }
    key = tuple(sorted(scalars.items()))
    if key not in _GRAPH_CACHE:
        _GRAPH_CACHE[key] = build_graph(scalars)
    nc = _GRAPH_CACHE[key]

    in_maps = [prep_core_inputs(c, shared) for c in range(NCORES)]
    res = run_bass_kernel_spmd(nc, in_maps, core_ids=list(range(NCORES)))
    global LAST_RESULTS
    LAST_RESULTS = res
    out = np.concatenate([res.results[c]["out"] for c in range(NCORES)], axis=0)
    return out.reshape(B, K, 1).astype(np.float32)


if __name__ == "__main__":
    g = build_graph({"a_src2": 0.01, "a_dst2": 0.02, "b2": 0.0, "c2": 0.1,
                     "ce_nz": False})
    print("graph built ok")


# revision 34
# speedup vs baseline: 1.0308x; 1.0176x over previous
"""Trainium2 Bass kernel for nn_Action_Decoder (GAT-based action decoder).

v3 strategy (8 NeuronCores, pure data-parallel over batch):
  - B=4096 sharded 8 x 512 samples/core; weights replicated; 4 tiles of
    128 samples on the partition dim.
  - Gather restructure: host packs a [BS*36, 7*128] bf16 block table
    (rows for (sample, substation) = 6 node rows + 1 sub row, built with
    sample-INDEPENDENT indexing from the fixed [36,6] element table).
    The device gathers ONE 1792-byte block per sample: 2 dma_gather ops
    of 256 idxs each (vs 8 ops / 3584 descriptors in v2) -> SWDGE
    descriptor generation drops ~7x and blocks land directly in x^T
    layout ([feature, k, sample]).
  - idx DMA issued first on the sync queue; consts spread across
    vector/tensor/scalar queues so nothing delays the gather.
  - Layer-1 on PE as v2 (obs folded via W_proj@W1a, shared obs+sub part
    broadcast over the 6 nodes with an identity-stationary matmul,
    fused transpose+e via the trident matmul).
  - Tail engine rebalance: ALL PSUM evictions + exps on Scalar, e_raw /
    Z1 / relu / one alpha*h product row on GpSimd, DVE keeps the
    irreducible tensor-tensor chain (apply + folds + elu-combine + W2).
  - elu fused: elu(x)+1 = min(exp(x),1) + relu(x), one scalar Exp + one
    DVE scalar_tensor_tensor; W2 dot via one mult + one axis-reduce.
"""

import os
import sys

import numpy as np

for _p in ("/root/.axon_site", "/root/.axon_site/_ro/trn_rl_repo",
           "/root/.axon_site/_ro/pypackages", "/opt/trn_rl_repo", "/opt/pypackages"):
    if os.path.isdir(_p) and _p not in sys.path:
        sys.path.append(_p)

import ml_dtypes

import concourse.bass as bass
import concourse.tile as tile
from concourse import bacc
from concourse import mybir
from concourse.bass_utils import run_bass_kernel_spmd

# Problem dims
B, N, S, K, H, OBS = 4096, 177, 36, 6, 128, 500
HEADS, FH = 4, 32
NCORES = 8
BS = B // NCORES          # 512 samples per core
NT = BS // 128            # 4 tiles of 128 samples
OBS_PAD = 512             # pad 500 -> 512
RB = 7                    # rows per (sample, substation) block

F32 = mybir.dt.float32
BF16 = mybir.dt.bfloat16
I16 = mybir.dt.int16
AX = mybir.AxisListType
OP = mybir.AluOpType
ACT = mybir.ActivationFunctionType

LRELU_SLOPE = 0.2
# GpSimd does SWDGE gathers ONLY. Measured: any gpsimd elementwise op
# running concurrently with DVE slows BOTH ~3-40x (shared SBUF port),
# and using gpsimd tensor ops forces a Pool ucode lib swap (~12us).


def build_graph(scalars):
    as2 = float(scalars["a_src2"])
    ad2 = float(scalars["a_dst2"])
    b2 = float(scalars["b2"])
    c2 = float(scalars["c2"])
    ce_nz = bool(scalars["ce_nz"])

    nc = bacc.Bacc(num_swdge_queues=4)

    table7 = nc.declare_dram_parameter("table7", [BS * S, RB * H], BF16,
                                       isOutput=False)
    idx2 = nc.declare_dram_parameter("idx2", [128, 32], I16, isOutput=False)
    obs_T = nc.declare_dram_parameter("obs_T", [128, 4, BS], BF16, isOutput=False)
    wfold = nc.declare_dram_parameter("wfold", [128, 4, H], BF16, isOutput=False)
    w1bc = nc.declare_dram_parameter("w1bc", [H, 2, H], BF16, isOutput=False)
    trident = nc.declare_dram_parameter("trident", [H, 136], BF16, isOutput=False)
    identd = nc.declare_dram_parameter("identd", [H, H], BF16, isOutput=False)
    biash = nc.declare_dram_parameter("biash", [H, 1], F32, isOutput=False)
    ce = nc.declare_dram_parameter("ce", [1, 48], F32, isOutput=False)
    w2r = nc.declare_dram_parameter("w2r", [1, H], BF16, isOutput=False)
    out_ext = nc.declare_dram_parameter("out", [BS, K], F32, isOutput=True)

    with tile.TileContext(nc) as tc:
        with (
            tc.tile_pool(name="consts", bufs=1) as consts,
            tc.tile_pool(name="gat", bufs=1) as gat,
            tc.tile_pool(name="ht", bufs=2) as htp,
            tc.tile_pool(name="work", bufs=1) as work,
            tc.tile_pool(name="psS", bufs=1, space="PSUM") as psS,
            tc.tile_pool(name="psH", bufs=2, space="PSUM") as psH,
            tc.tile_pool(name="psT", bufs=1, space="PSUM") as psT,
        ):
            # ---- idx first (the gathers' only dependency), on sync ----
            idx_sb = consts.tile([128, 32], I16)
            nc.sync.dma_start(out=idx_sb, in_=idx2[:, :])

            # ---- consts spread over the sync + scalar queues ----
            obs_sb = consts.tile([128, 4, BS], BF16)
            nc.sync.dma_start(out=obs_sb[:, 0:2, :], in_=obs_T[:, 0:2, :])
            nc.scalar.dma_start(out=obs_sb[:, 2:4, :], in_=obs_T[:, 2:4, :])
            wfold_sb = consts.tile([128, 4, 128], BF16)
            nc.scalar.dma_start(out=wfold_sb[:, :, :], in_=wfold[:, :, :])
            w1bc_sb = consts.tile([128, 2, 128], BF16)
            nc.sync.dma_start(out=w1bc_sb[:, :, :], in_=w1bc[:, :, :])
            tri_sb = consts.tile([128, 136], BF16)
            nc.sync.dma_start(out=tri_sb, in_=trident[:, :])
            ident_sb = consts.tile([128, 128], BF16)
            nc.sync.dma_start(out=ident_sb, in_=identd[:, :])
            biash_sb = consts.tile([128, 1], F32)
            nc.scalar.dma_start(out=biash_sb, in_=biash[:, :])
            w2_sb = consts.tile([128, 128], BF16)
            nc.scalar.dma_start(
                out=w2_sb,
                in_=bass.AP(tensor=w2r, offset=0, ap=[[0, 128], [1, 128]]),
            )
            if ce_nz:
                ce_sb = consts.tile([128, 48], F32)
                nc.scalar.dma_start(
                    out=ce_sb,
                    in_=bass.AP(tensor=ce, offset=0, ap=[[0, 128], [1, 48]]),
                )

            # ---- gathers: one 1792B block per sample, 128 idx per TILE
            # (4 gathers) so tile 0's data lands as early as possible; the
            # first SWDGE instruction absorbs the Pool ucode lib-load
            # latency (~11.5us after MODIFY_POOL_CONFIG). ----
            gs = []
            for t in range(NT):
                g = gat.tile([128, RB, 128], BF16, tag=f"g{t}")
                nc.gpsimd.dma_gather(
                    out_ap=g[:, :, :],
                    in_ap=table7[:, :],
                    idxs_ap=idx_sb[:, t * 8:(t + 1) * 8],
                    num_idxs=128, num_idxs_reg=128, elem_size=RB * H,
                    transpose=True, queue_num=t,
                )
                gs.append(g)

            # ---- shared part: obs (4 accumulating chunks over all 512
            # samples) + per-tile sub row. First psS write must be the only
            # start=True on that bank. ----
            shF = psS.tile([128, 512], F32)
            for c in range(4):
                nc.tensor.matmul(shF[:, :], wfold_sb[:, c, :], obs_sb[:, c, :],
                                 start=(c == 0), stop=False)

            sh_sb = work.tile([128, 4, 128], BF16, tag="sh")
            hbL = work.tile([128, NT, 768], BF16, tag="hbL")
            e_sd = work.tile([128, NT, 48], F32, tag="esd")
            e4 = e_sd[:, :, :].rearrange("p t (k s h) -> p t k s h", s=2, h=4)
            e_raw = work.tile([128, NT, 144], BF16, tag="eraw")
            e_rawf = e_raw[:, :, :].rearrange("p t x -> p (t x)")
            e_lrf = work.tile([128, NT * 144], BF16, tag="elr")
            E1 = work.tile([128, NT * 144], F32, tag="E1")
            Z1 = work.tile([128, 96], F32, tag="Z1")
            rZ1 = work.tile([128, 96], F32, tag="rZ1")
            al = work.tile([128, NT, 144], BF16, tag="al")
            prod = work.tile([128, 24, 768], BF16, tag="prod")
            f1 = work.tile([128, 24, 384], BF16, tag="f1")
            f2 = work.tile([128, 24, 128], BF16, tag="f2")
            attn = work.tile([128, 24, 128], BF16, tag="attn")
            minat = work.tile([128, 24, 128], BF16, tag="minat")
            expm = work.tile([128, 24, 128], BF16, tag="expm")
            relux = work.tile([128, 24, 128], BF16, tag="relux")
            v1 = work.tile([128, 24, 128], BF16, tag="v1")
            vw = work.tile([128, 24, 128], BF16, tag="vw")
            vwa = work.tile([128, 24, 64], BF16, tag="vwa")
            vwb = work.tile([128, 24, 32], BF16, tag="vwb")
            h2 = work.tile([128, 24], F32, tag="h2")
            h2c = work.tile([128, 24], F32, tag="h2c")
            h2s = work.tile([128, 24], F32, tag="h2s")
            h2d = work.tile([128, 24], F32, tag="h2d")
            e2_raw = work.tile([128, 144], F32, tag="e2raw")
            e2_lr = work.tile([128, 144], F32, tag="e2lr")
            E2 = work.tile([128, 144], F32, tag="E2")
            Z2 = work.tile([128, 24], F32, tag="Z2")
            rZ2 = work.tile([128, 24], F32, tag="rZ2")
            P2 = work.tile([128, 144], F32, tag="P2")
            S2 = work.tile([128, 24], F32, tag="S2")
            out_sb = work.tile([128, 24], F32, tag="outsb")

            def pe_pair(t0):
                # paired emission: same-stationary matmuls for both tiles
                # back to back (1 LDWEIGHTS per weight instead of per tile)
                hps = {}
                for t in (t0, t0 + 1):
                    g = gs[t]
                    ts = slice(t * 128, (t + 1) * 128)
                    nc.tensor.matmul(shF[:, ts], w1bc_sb[:, 0, :],
                                     g[:, 6, :], start=False,
                                     stop=True)
                    nc.scalar.activation(sh_sb[:, t, :], shF[:, ts], ACT.Copy)
                for t in (t0, t0 + 1):
                    g = gs[t]
                    h_ps = psH.tile([128, 768], F32, tag="hps")
                    hps[t] = h_ps
                    nc.tensor.matmul(h_ps[:, 0:512], w1bc_sb[:, 1, :],
                                     g[:, 0:4, :], start=True,
                                     stop=False)
                    nc.tensor.matmul(h_ps[:, 512:768], w1bc_sb[:, 1, :],
                                     g[:, 4:6, :], start=True,
                                     stop=False)
                for t in (t0, t0 + 1):
                    h_ps = hps[t]
                    for hs, nk in ((slice(0, 512), 4), (slice(512, 768), 2)):
                        mov = (sh_sb[:, t, :].unsqueeze(1)
                               .broadcast_to([128, nk, 128]))
                        nc.tensor.matmul(h_ps[:, hs], ident_sb[:, :], mov,
                                         start=False, stop=True)
                for t in (t0, t0 + 1):
                    pe_tri(t, hps[t])

            def pe_tri(t, h_ps):
                hT = htp.tile([128, 768], BF16, tag="hT")
                if t < 2:
                    # vector idles before the half-0 tail; scalar's serial
                    # eviction chain gates E1 -> do t0/t1's hT there
                    nc.vector.tensor_scalar(hT[:, :], h_ps[:, :],
                                            biash_sb[:, 0:1], None, OP.add)
                else:
                    nc.scalar.activation(hT[:, :], h_ps[:, :], ACT.Identity,
                                         bias=biash_sb[:, :], scale=1.0)
                # fused transpose + e via trident; 3+3 split keeps every
                # matmul output inside one PSUM bank.
                ps2a = psT.tile([128, 3, 136], F32, tag="ps2a")
                ps2b = psT.tile([128, 3, 136], F32, tag="ps2b")
                for k in range(K):
                    ps2k = ps2a[:, k, :] if k < 3 else ps2b[:, k - 3, :]
                    nc.tensor.matmul(
                        ps2k, hT[:, k * 128:(k + 1) * 128],
                        tri_sb[:, :], start=True, stop=True,
                    )
                # evictions: hbL casts on scalar, tiny e copies on vector
                nc.scalar.activation(
                    hbL[:, t, 0:384].rearrange("p (k f) -> p k f", k=3),
                    ps2a[:, :, 0:128], ACT.Copy)
                if t < 2:
                    nc.vector.tensor_copy(
                        hbL[:, t, 384:768].rearrange("p (k f) -> p k f", k=3),
                        ps2b[:, :, 0:128])
                else:
                    nc.scalar.activation(
                        hbL[:, t, 384:768].rearrange("p (k f) -> p k f", k=3),
                        ps2b[:, :, 0:128], ACT.Copy)
                nc.vector.tensor_copy(
                    e_sd[:, t, 0:24].rearrange("p (k s) -> p k s", k=3),
                    ps2a[:, :, 128:136])
                nc.vector.tensor_copy(
                    e_sd[:, t, 24:48].rearrange("p (k s) -> p k s", k=3),
                    ps2b[:, :, 128:136])
                if ce_nz:
                    nc.vector.tensor_tensor(
                        e_sd[:, t, :], e_sd[:, t, :], ce_sb[:, :], OP.subtract)

            def softmax_tile(t):
                # per-tile softmax chain: lets tile t's apply products
                # start as soon as ITS alpha is ready (latency-critical
                # for the first half; half-1 overlaps DVE work anyway)
                st = slice(t * 144, (t + 1) * 144)
                zt = slice(t * 24, (t + 1) * 24)
                nc.vector.tensor_tensor(
                    e_raw[:, t, :].rearrange("p (i j h) -> p i j h",
                                             j=6, h=4),
                    e4[:, t, :, 1, :].unsqueeze(2)
                    .broadcast_to([128, 6, 6, 4]),
                    e4[:, t, :, 0, :].unsqueeze(1)
                    .broadcast_to([128, 6, 6, 4]),
                    OP.add,
                )
                nc.vector.scalar_tensor_tensor(
                    e_lrf[:, st], e_rawf[:, st], LRELU_SLOPE, e_rawf[:, st],
                    OP.mult, OP.max)
                nc.scalar.activation(E1[:, st], e_lrf[:, st], ACT.Exp)
                nc.vector.tensor_reduce(
                    Z1[:, zt],
                    E1[:, st].rearrange("p (i j h) -> p i j h", j=6, h=4)
                    .transpose([0, 1, 3, 2]),
                    axis=AX.X, op=OP.add)
                nc.vector.reciprocal_approx_fast(rZ1[:, zt], Z1[:, zt])
                nc.vector.tensor_tensor(
                    al[:, t, :].rearrange("p (i j h) -> p i j h", j=6, h=4),
                    E1[:, st].rearrange("p (i j h) -> p i j h", j=6, h=4),
                    rZ1[:, zt].rearrange("p (i h) -> p i h", h=4)
                    .unsqueeze(2).broadcast_to([128, 6, 6, 4]),
                    OP.mult,
                )
                for i in range(K):
                    nc.vector.tensor_tensor(
                        prod[:, t * 6 + i, :].rearrange(
                            "p (j f h) -> p j f h", f=32, h=4),
                        al[:, t, i * 24:(i + 1) * 24]
                        .rearrange("p (j h) -> p j h", h=4)
                        .unsqueeze(2).broadcast_to([128, 6, 32, 4]),
                        hbL[:, t, :].rearrange("p (j f h) -> p j f h",
                                               f=32, h=4),
                        OP.mult,
                    )

            def tail_half(half):
                th0 = 2 * half
                cs = slice(th0 * 6, (th0 + 2) * 6)
                softmax_tile(th0)
                softmax_tile(th0 + 1)
                nc.vector.tensor_add(f1[:, cs, :], prod[:, cs, 0:384],
                                     prod[:, cs, 384:768])
                nc.vector.tensor_add(f2[:, cs, :], f1[:, cs, 0:128],
                                     f1[:, cs, 128:256])
                nc.vector.tensor_add(attn[:, cs, :], f2[:, cs, :],
                                     f1[:, cs, 256:384])
                # elu(x)+1 = exp(min(x,0)) + relu(x)  (-1 folded into c2);
                # min/relu at 4x tensor_scalar rate, exp on scalar.
                nc.vector.tensor_scalar_min(minat[:, cs, :], attn[:, cs, :],
                                            0.0)
                nc.vector.tensor_scalar_max(relux[:, cs, :], attn[:, cs, :],
                                            0.0)

            def tail_b(half):
                cs = slice(2 * half * 6, (2 * half + 2) * 6)
                for t in (2 * half, 2 * half + 1):
                    c6 = slice(t * 6, (t + 1) * 6)
                    nc.scalar.activation(expm[:, c6, :], minat[:, c6, :],
                                         ACT.Exp)
                    nc.vector.tensor_add(v1[:, c6, :], expm[:, c6, :],
                                         relux[:, c6, :])
                nc.vector.tensor_tensor(
                    vw[:, cs, :], v1[:, cs, :],
                    w2_sb[:, :].unsqueeze(1).broadcast_to([128, 12, 128]),
                    OP.mult)
                # fold W2-dot with cheap 2x adds, then a short 1x reduce
                nc.vector.tensor_add(vwa[:, cs, :], vw[:, cs, 0:64],
                                     vw[:, cs, 64:128])
                nc.vector.tensor_add(vwb[:, cs, :], vwa[:, cs, 0:32],
                                     vwa[:, cs, 32:64])
                nc.vector.tensor_reduce(h2[:, cs], vwb[:, cs, :], axis=AX.X,
                                        op=OP.add)

            def l2_half(half):
                # layer-2 GAT attention over the 6 nodes, per tile pair
                cs = slice(half * 12, (half + 1) * 12)
                nc.vector.tensor_scalar(h2c[:, cs], h2[:, cs], -c2, None,
                                        OP.add)
                nc.vector.tensor_scalar(h2s[:, cs], h2[:, cs], as2,
                                        -c2 * (as2 + ad2), OP.mult, OP.add)
                nc.vector.tensor_scalar(h2d[:, cs], h2[:, cs], ad2, None,
                                        OP.mult)
                h2dv = h2d[:, cs].rearrange("p (t i) -> p t i", t=2)
                h2sv = h2s[:, cs].rearrange("p (t j) -> p t j", t=2)
                h2cv = h2c[:, cs].rearrange("p (t j) -> p t j", t=2)
                es = slice(half * 72, (half + 1) * 72)
                nc.vector.tensor_tensor(
                    e2_raw[:, es].rearrange("p (t i j) -> p t i j", t=2, j=6),
                    h2dv.unsqueeze(3).broadcast_to([128, 2, 6, 6]),
                    h2sv.unsqueeze(2).broadcast_to([128, 2, 6, 6]),
                    OP.add,
                )
                nc.vector.scalar_tensor_tensor(
                    e2_lr[:, es], e2_raw[:, es], LRELU_SLOPE, e2_raw[:, es],
                    OP.mult, OP.max)
                nc.scalar.activation(E2[:, es], e2_lr[:, es], ACT.Exp)
                nc.vector.tensor_reduce(
                    Z2[:, cs],
                    E2[:, es].rearrange("p (ti j) -> p ti j", j=6),
                    axis=AX.X, op=OP.add)
                nc.vector.reciprocal_approx_fast(rZ2[:, cs], Z2[:, cs])
                nc.vector.tensor_tensor(
                    P2[:, es].rearrange("p (t i j) -> p t i j", t=2, j=6),
                    E2[:, es].rearrange("p (t i j) -> p t i j", t=2, j=6),
                    h2cv.unsqueeze(2).broadcast_to([128, 2, 6, 6]),
                    OP.mult,
                )
                nc.vector.tensor_reduce(
                    S2[:, cs],
                    P2[:, es].rearrange("p (ti j) -> p ti j", j=6),
                    axis=AX.X, op=OP.add,
                )
                nc.vector.tensor_mul(out_sb[:, cs], S2[:, cs], rZ2[:, cs])
                if b2 != 0.0:
                    nc.vector.tensor_scalar(out_sb[:, cs], out_sb[:, cs], b2,
                                            None, OP.add)
                nc.sync.dma_start(
                    out=bass.AP(tensor=out_ext, offset=half * 2 * 128 * K,
                                ap=[[K, 128], [128 * K, 2], [1, K]]),
                    in_=out_sb[:, cs].rearrange("p (t k) -> p t k", t=2))

            # emission order = per-engine program order: keep each half's
            # tail right after its two tiles so no engine's queue blocks
            # half-0 work behind tile-2/3 dependencies.
            pe_pair(0)
            tail_half(0)
            pe_pair(2)
            tail_b(0)
            l2_half(0)
            tail_half(1)
            tail_b(1)
            l2_half(1)

    nc.finalize()
    return nc


def prep_shared(inp):
    """Host-side layout prep shared across cores (index math / weight
    folding / dtype casts only -- all tensor FLOPs on the batch stay on
    device)."""
    bf = ml_dtypes.bfloat16
    node = np.asarray(inp["node_embeddings"], np.float32).astype(bf)
    sub = np.asarray(inp["substation_embeddings"], np.float32).astype(bf)
    elem = np.asarray(inp["sub_id_to_elem_id"], np.int64)
    tbl = np.empty((B, S, RB, H), bf)
    tbl[:, :, 0:K, :] = node[:, elem.reshape(-1), :].reshape(B, S, K, H)
    tbl[:, :, 6, :] = sub
    sub_idx = np.asarray(inp["sub_choice"], np.int64).reshape(B)
    obs = np.asarray(inp["org_obs"], np.float32)

    W1 = np.asarray(inp["W1"], np.float32)
    W1a, W1b, W1c = W1[0:H], W1[H:2 * H], W1[2 * H:3 * H]
    Wp = np.asarray(inp["W_proj"], np.float32)
    wfold = np.zeros((OBS_PAD, H), np.float32)
    wfold[:OBS] = Wp @ W1a
    wfold4 = wfold.reshape(4, 128, H).transpose(1, 0, 2).copy()
    bias_h = (np.asarray(inp["b1"], np.float32)
              + np.asarray(inp["b_proj"], np.float32) @ W1a)

    a_src1 = np.asarray(inp["a_src1"], np.float32)
    a_dst1 = np.asarray(inp["a_dst1"], np.float32)
    asrc_m = np.zeros((H, 8), np.float32)
    for h in range(HEADS):
        asrc_m[h * FH:(h + 1) * FH, h] = a_src1[h]
        asrc_m[h * FH:(h + 1) * FH, 4 + h] = a_dst1[h]
    # first 128 cols: permutation (h,f)->(f,h) so the transpose emits the
    # apply-friendly layout directly
    perm = np.zeros((H, H), np.float32)
    for h in range(HEADS):
        for f in range(FH):
            perm[h * FH + f, f * HEADS + h] = 1.0
    trident = np.concatenate([perm, asrc_m], axis=1)

    # e must be computed from h WITHOUT b1 (but WITH the obs-projection
    # bias, which is part of the reference h) -> correct only for b1.
    bh = np.asarray(inp["b1"], np.float32).reshape(HEADS, FH)
    cek = np.zeros((K, 8), np.float32)
    cek[:, 0:4] = (bh * a_src1).sum(-1)[None, :]
    cek[:, 4:8] = (bh * a_dst1).sum(-1)[None, :]

    shared = {
        "tbl": tbl,
        "sub_idx": sub_idx,
        "obs": obs,
        "wfold": wfold4.astype(bf),
        "w1bc": np.stack([W1b, W1c], axis=1).astype(bf),
        "trident": trident.astype(bf),
        "biash": bias_h.reshape(H, 1).astype(np.float32),
        "ce": cek.reshape(1, 48).astype(np.float32),
        # W2 permuted to the (f,h) feature order used by the apply layout
        "w2r": np.asarray(inp["W2"], np.float32).reshape(HEADS, FH).T
               .reshape(1, H).astype(bf),
        "identd": np.eye(H, dtype=np.float32).astype(bf),
    }
    return shared


def prep_core_inputs(core, shared):
    bf = ml_dtypes.bfloat16
    s = slice(core * BS, (core + 1) * BS)

    obs_T = np.zeros((OBS_PAD, BS), np.float32)
    obs_T[:OBS, :] = shared["obs"][s].T
    obs_T4 = obs_T.reshape(4, 128, BS).transpose(1, 0, 2).copy()

    sub_idx = shared["sub_idx"][s]
    idx_cols = []
    for t in range(NT):
        bl = np.arange(128) + t * 128
        v = (bl * S + sub_idx[bl]).astype(np.int16)
        blk = v.reshape(8, 16).T
        idx_cols.append(np.tile(blk, (8, 1)))
    idx2 = np.ascontiguousarray(np.concatenate(idx_cols, axis=1))

    return {
        "table7": shared["tbl"][s].reshape(BS * S, RB * H),
        "idx2": idx2,
        "obs_T": obs_T4.astype(bf),
        "wfold": shared["wfold"],
        "w1bc": shared["w1bc"],
        "trident": shared["trident"],
        "biash": shared["biash"],
        "ce": shared["ce"],
        "w2r": shared["w2r"],
        "identd": shared["identd"],
    }


_GRAPH_CACHE = {}
LAST_RESULTS = None


def kernel(**inputs):
    inp = {k: np.asarray(v) for k, v in inputs.items()}
    W2 = np.asarray(inp["W2"], np.float32)
    shared = prep_shared(inp)
    scalars = {
        "a_src2": float(np.asarray(inp["a_src2"]).reshape(-1)[0]),
        "a_dst2": float(np.asarray(inp["a_dst2"]).reshape(-1)[0]),
        "b2": float(np.asarray(inp["b2"]).reshape(-1)[0]),
        "c2": float(W2.sum()),
        "ce_nz": bool(np.any(np.abs(shared["ce"]) > 0)),
    }
    key = tuple(sorted(scalars.items()))
    if key not in _GRAPH_CACHE:
        _GRAPH_CACHE[key] = build_graph(scalars)
    nc = _GRAPH_CACHE[key]

    in_maps = [prep_core_inputs(c, shared) for c in range(NCORES)]
    res = run_bass_kernel_spmd(nc, in_maps, core_ids=list(range(NCORES)))
    global LAST_RESULTS
    LAST_RESULTS = res
    out = np.concatenate([res.results[c]["out"] for c in range(NCORES)], axis=0)
    return out.reshape(B, K, 1).astype(np.float32)


if __name__ == "__main__":
    g = build_graph({"a_src2": 0.01, "a_dst2": 0.02, "b2": 0.0, "c2": 0.1,
                     "ce_nz": False})
    print("graph built ok")


# revision 35
# speedup vs baseline: 1.0313x; 1.0005x over previous
"""Trainium2 Bass kernel for nn_Action_Decoder (GAT-based action decoder).

v3 strategy (8 NeuronCores, pure data-parallel over batch):
  - B=4096 sharded 8 x 512 samples/core; weights replicated; 4 tiles of
    128 samples on the partition dim.
  - Gather restructure: host packs a [BS*36, 7*128] bf16 block table
    (rows for (sample, substation) = 6 node rows + 1 sub row, built with
    sample-INDEPENDENT indexing from the fixed [36,6] element table).
    The device gathers ONE 1792-byte block per sample: 2 dma_gather ops
    of 256 idxs each (vs 8 ops / 3584 descriptors in v2) -> SWDGE
    descriptor generation drops ~7x and blocks land directly in x^T
    layout ([feature, k, sample]).
  - idx DMA issued first on the sync queue; consts spread across
    vector/tensor/scalar queues so nothing delays the gather.
  - Layer-1 on PE as v2 (obs folded via W_proj@W1a, shared obs+sub part
    broadcast over the 6 nodes with an identity-stationary matmul,
    fused transpose+e via the trident matmul).
  - Tail engine rebalance: ALL PSUM evictions + exps on Scalar, e_raw /
    Z1 / relu / one alpha*h product row on GpSimd, DVE keeps the
    irreducible tensor-tensor chain (apply + folds + elu-combine + W2).
  - elu fused: elu(x)+1 = min(exp(x),1) + relu(x), one scalar Exp + one
    DVE scalar_tensor_tensor; W2 dot via one mult + one axis-reduce.
"""

import os
import sys

import numpy as np

for _p in ("/root/.axon_site", "/root/.axon_site/_ro/trn_rl_repo",
           "/root/.axon_site/_ro/pypackages", "/opt/trn_rl_repo", "/opt/pypackages"):
    if os.path.isdir(_p) and _p not in sys.path:
        sys.path.append(_p)

import ml_dtypes

import concourse.bass as bass
import concourse.tile as tile
from concourse import bacc
from concourse import mybir
from concourse.bass_utils import run_bass_kernel_spmd

# Problem dims
B, N, S, K, H, OBS = 4096, 177, 36, 6, 128, 500
HEADS, FH = 4, 32
NCORES = 8
BS = B // NCORES          # 512 samples per core
NT = BS // 128            # 4 tiles of 128 samples
OBS_PAD = 512             # pad 500 -> 512
RB = 7                    # rows per (sample, substation) block

F32 = mybir.dt.float32
BF16 = mybir.dt.bfloat16
I16 = mybir.dt.int16
AX = mybir.AxisListType
OP = mybir.AluOpType
ACT = mybir.ActivationFunctionType

LRELU_SLOPE = 0.2
# GpSimd does SWDGE gathers ONLY. Measured: any gpsimd elementwise op
# running concurrently with DVE slows BOTH ~3-40x (shared SBUF port),
# and using gpsimd tensor ops forces a Pool ucode lib swap (~12us).


def build_graph(scalars):
    as2 = float(scalars["a_src2"])
    ad2 = float(scalars["a_dst2"])
    b2 = float(scalars["b2"])
    c2 = float(scalars["c2"])
    ce_nz = bool(scalars["ce_nz"])

    nc = bacc.Bacc(num_swdge_queues=4)

    table7 = nc.declare_dram_parameter("table7", [BS * S, RB * H], BF16,
                                       isOutput=False)
    idx2 = nc.declare_dram_parameter("idx2", [128, 32], I16, isOutput=False)
    obs_T = nc.declare_dram_parameter("obs_T", [128, 4, BS], BF16, isOutput=False)
    wfold = nc.declare_dram_parameter("wfold", [128, 4, H], BF16, isOutput=False)
    w1bc = nc.declare_dram_parameter("w1bc", [H, 2, H], BF16, isOutput=False)
    trident = nc.declare_dram_parameter("trident", [H, 136], BF16, isOutput=False)
    identd = nc.declare_dram_parameter("identd", [H, H], BF16, isOutput=False)
    biash = nc.declare_dram_parameter("biash", [H, 1], F32, isOutput=False)
    ce = nc.declare_dram_parameter("ce", [1, 48], F32, isOutput=False)
    w2r = nc.declare_dram_parameter("w2r", [1, H], BF16, isOutput=False)
    out_ext = nc.declare_dram_parameter("out", [BS, K], F32, isOutput=True)

    with tile.TileContext(nc) as tc:
        with (
            tc.tile_pool(name="consts", bufs=1) as consts,
            tc.tile_pool(name="gat", bufs=1) as gat,
            tc.tile_pool(name="ht", bufs=2) as htp,
            tc.tile_pool(name="work", bufs=1) as work,
            tc.tile_pool(name="psS", bufs=1, space="PSUM") as psS,
            tc.tile_pool(name="psH", bufs=2, space="PSUM") as psH,
            tc.tile_pool(name="psT", bufs=1, space="PSUM") as psT,
        ):
            # ---- idx first (the gathers' only dependency), on sync ----
            idx_sb = consts.tile([128, 32], I16)
            nc.sync.dma_start(out=idx_sb, in_=idx2[:, :])

            # ---- consts spread over the sync + scalar queues ----
            obs_sb = consts.tile([128, 4, BS], BF16)
            nc.sync.dma_start(out=obs_sb[:, 0:2, :], in_=obs_T[:, 0:2, :])
            nc.scalar.dma_start(out=obs_sb[:, 2:4, :], in_=obs_T[:, 2:4, :])
            wfold_sb = consts.tile([128, 4, 128], BF16)
            nc.scalar.dma_start(out=wfold_sb[:, :, :], in_=wfold[:, :, :])
            w1bc_sb = consts.tile([128, 2, 128], BF16)
            nc.sync.dma_start(out=w1bc_sb[:, :, :], in_=w1bc[:, :, :])
            tri_sb = consts.tile([128, 136], BF16)
            nc.sync.dma_start(out=tri_sb, in_=trident[:, :])
            ident_sb = consts.tile([128, 128], BF16)
            nc.sync.dma_start(out=ident_sb, in_=identd[:, :])
            biash_sb = consts.tile([128, 1], F32)
            nc.scalar.dma_start(out=biash_sb, in_=biash[:, :])
            w2_sb = consts.tile([128, 128], BF16)
            nc.scalar.dma_start(
                out=w2_sb,
                in_=bass.AP(tensor=w2r, offset=0, ap=[[0, 128], [1, 128]]),
            )
            if ce_nz:
                ce_sb = consts.tile([128, 48], F32)
                nc.scalar.dma_start(
                    out=ce_sb,
                    in_=bass.AP(tensor=ce, offset=0, ap=[[0, 128], [1, 48]]),
                )

            # ---- gathers: one 1792B block per sample, 128 idx per TILE
            # (4 gathers) so tile 0's data lands as early as possible; the
            # first SWDGE instruction absorbs the Pool ucode lib-load
            # latency (~11.5us after MODIFY_POOL_CONFIG). ----
            gs = []
            for t in range(NT):
                g = gat.tile([128, RB, 128], BF16, tag=f"g{t}")
                nc.gpsimd.dma_gather(
                    out_ap=g[:, :, :],
                    in_ap=table7[:, :],
                    idxs_ap=idx_sb[:, t * 8:(t + 1) * 8],
                    num_idxs=128, num_idxs_reg=128, elem_size=RB * H,
                    transpose=True, queue_num=t,
                )
                gs.append(g)

            # ---- shared part: obs (4 accumulating chunks over all 512
            # samples) + per-tile sub row. First psS write must be the only
            # start=True on that bank. ----
            shF = psS.tile([128, 512], F32)
            for c in range(4):
                nc.tensor.matmul(shF[:, :], wfold_sb[:, c, :], obs_sb[:, c, :],
                                 start=(c == 0), stop=False)

            sh_sb = work.tile([128, 4, 128], BF16, tag="sh")
            hbL = work.tile([128, NT, 768], BF16, tag="hbL")
            e_sd = work.tile([128, NT, 48], F32, tag="esd")
            e4 = e_sd[:, :, :].rearrange("p t (k s h) -> p t k s h", s=2, h=4)
            e_raw = work.tile([128, NT, 144], BF16, tag="eraw")
            e_rawf = e_raw[:, :, :].rearrange("p t x -> p (t x)")
            e_lrf = work.tile([128, NT * 144], BF16, tag="elr")
            E1 = work.tile([128, NT * 144], F32, tag="E1")
            Z1 = work.tile([128, 96], F32, tag="Z1")
            rZ1 = work.tile([128, 96], F32, tag="rZ1")
            al = work.tile([128, NT, 144], BF16, tag="al")
            prod = work.tile([128, 24, 768], BF16, tag="prod")
            f1 = work.tile([128, 24, 384], BF16, tag="f1")
            f2 = work.tile([128, 24, 128], BF16, tag="f2")
            attn = work.tile([128, 24, 128], BF16, tag="attn")
            minat = work.tile([128, 24, 128], BF16, tag="minat")
            expm = work.tile([128, 24, 128], BF16, tag="expm")
            relux = work.tile([128, 24, 128], BF16, tag="relux")
            v1 = work.tile([128, 24, 128], BF16, tag="v1")
            vw = work.tile([128, 24, 128], BF16, tag="vw")
            vwa = work.tile([128, 24, 64], BF16, tag="vwa")
            vwb = work.tile([128, 24, 32], BF16, tag="vwb")
            h2 = work.tile([128, 24], F32, tag="h2")
            h2c = work.tile([128, 24], F32, tag="h2c")
            h2s = work.tile([128, 24], F32, tag="h2s")
            h2d = work.tile([128, 24], F32, tag="h2d")
            e2_raw = work.tile([128, 144], F32, tag="e2raw")
            e2_lr = work.tile([128, 144], F32, tag="e2lr")
            E2 = work.tile([128, 144], F32, tag="E2")
            Z2 = work.tile([128, 24], F32, tag="Z2")
            rZ2 = work.tile([128, 24], F32, tag="rZ2")
            P2 = work.tile([128, 144], F32, tag="P2")
            S2 = work.tile([128, 24], F32, tag="S2")
            out_sb = work.tile([128, 24], F32, tag="outsb")

            def pe_pair(t0):
                # paired emission: same-stationary matmuls for both tiles
                # back to back (1 LDWEIGHTS per weight instead of per tile)
                hps = {}
                for t in (t0, t0 + 1):
                    g = gs[t]
                    ts = slice(t * 128, (t + 1) * 128)
                    nc.tensor.matmul(shF[:, ts], w1bc_sb[:, 0, :],
                                     g[:, 6, :], start=False,
                                     stop=True)
                    nc.scalar.activation(sh_sb[:, t, :], shF[:, ts], ACT.Copy)
                for t in (t0, t0 + 1):
                    g = gs[t]
                    h_ps = psH.tile([128, 768], F32, tag="hps")
                    hps[t] = h_ps
                    nc.tensor.matmul(h_ps[:, 0:512], w1bc_sb[:, 1, :],
                                     g[:, 0:4, :], start=True,
                                     stop=False)
                    nc.tensor.matmul(h_ps[:, 512:768], w1bc_sb[:, 1, :],
                                     g[:, 4:6, :], start=True,
                                     stop=False)
                for t in (t0, t0 + 1):
                    h_ps = hps[t]
                    for hs, nk in ((slice(0, 512), 4), (slice(512, 768), 2)):
                        mov = (sh_sb[:, t, :].unsqueeze(1)
                               .broadcast_to([128, nk, 128]))
                        nc.tensor.matmul(h_ps[:, hs], ident_sb[:, :], mov,
                                         start=False, stop=True)
                for t in (t0, t0 + 1):
                    pe_tri(t, hps[t])

            def pe_tri(t, h_ps):
                hT = htp.tile([128, 768], BF16, tag="hT")
                if t < 2:
                    # vector idles before the half-0 tail; scalar's serial
                    # eviction chain gates E1 -> do t0/t1's hT there
                    nc.vector.tensor_scalar(hT[:, :], h_ps[:, :],
                                            biash_sb[:, 0:1], None, OP.add)
                else:
                    nc.scalar.activation(hT[:, :], h_ps[:, :], ACT.Identity,
                                         bias=biash_sb[:, :], scale=1.0)
                # fused transpose + e via trident; 3+3 split keeps every
                # matmul output inside one PSUM bank.
                ps2a = psT.tile([128, 3, 136], F32, tag="ps2a")
                ps2b = psT.tile([128, 3, 136], F32, tag="ps2b")
                for k in range(K):
                    ps2k = ps2a[:, k, :] if k < 3 else ps2b[:, k - 3, :]
                    nc.tensor.matmul(
                        ps2k, hT[:, k * 128:(k + 1) * 128],
                        tri_sb[:, :], start=True, stop=True,
                    )
                # evictions: hbL casts on scalar, tiny e copies on vector
                nc.scalar.activation(
                    hbL[:, t, 0:384].rearrange("p (k f) -> p k f", k=3),
                    ps2a[:, :, 0:128], ACT.Copy)
                if t < 2:
                    nc.vector.tensor_copy(
                        hbL[:, t, 384:768].rearrange("p (k f) -> p k f", k=3),
                        ps2b[:, :, 0:128])
                else:
                    nc.scalar.activation(
                        hbL[:, t, 384:768].rearrange("p (k f) -> p k f", k=3),
                        ps2b[:, :, 0:128], ACT.Copy)
                ecp_eng = nc.vector if t < 2 else nc.scalar
                if t < 2:
                    nc.vector.tensor_copy(
                        e_sd[:, t, 0:24].rearrange("p (k s) -> p k s", k=3),
                        ps2a[:, :, 128:136])
                    nc.vector.tensor_copy(
                        e_sd[:, t, 24:48].rearrange("p (k s) -> p k s", k=3),
                        ps2b[:, :, 128:136])
                else:
                    nc.scalar.activation(
                        e_sd[:, t, 0:24].rearrange("p (k s) -> p k s", k=3),
                        ps2a[:, :, 128:136], ACT.Copy)
                    nc.scalar.activation(
                        e_sd[:, t, 24:48].rearrange("p (k s) -> p k s", k=3),
                        ps2b[:, :, 128:136], ACT.Copy)
                if ce_nz:
                    nc.vector.tensor_tensor(
                        e_sd[:, t, :], e_sd[:, t, :], ce_sb[:, :], OP.subtract)

            def softmax_tile(t):
                # per-tile softmax chain: lets tile t's apply products
                # start as soon as ITS alpha is ready (latency-critical
                # for the first half; half-1 overlaps DVE work anyway)
                st = slice(t * 144, (t + 1) * 144)
                zt = slice(t * 24, (t + 1) * 24)
                nc.vector.tensor_tensor(
                    e_raw[:, t, :].rearrange("p (i j h) -> p i j h",
                                             j=6, h=4),
                    e4[:, t, :, 1, :].unsqueeze(2)
                    .broadcast_to([128, 6, 6, 4]),
                    e4[:, t, :, 0, :].unsqueeze(1)
                    .broadcast_to([128, 6, 6, 4]),
                    OP.add,
                )
                nc.vector.scalar_tensor_tensor(
                    e_lrf[:, st], e_rawf[:, st], LRELU_SLOPE, e_rawf[:, st],
                    OP.mult, OP.max)
                nc.scalar.activation(E1[:, st], e_lrf[:, st], ACT.Exp)
                nc.vector.tensor_reduce(
                    Z1[:, zt],
                    E1[:, st].rearrange("p (i j h) -> p i j h", j=6, h=4)
                    .transpose([0, 1, 3, 2]),
                    axis=AX.X, op=OP.add)
                nc.vector.reciprocal_approx_fast(rZ1[:, zt], Z1[:, zt])
                nc.vector.tensor_tensor(
                    al[:, t, :].rearrange("p (i j h) -> p i j h", j=6, h=4),
                    E1[:, st].rearrange("p (i j h) -> p i j h", j=6, h=4),
                    rZ1[:, zt].rearrange("p (i h) -> p i h", h=4)
                    .unsqueeze(2).broadcast_to([128, 6, 6, 4]),
                    OP.mult,
                )
                for i in range(K):
                    nc.vector.tensor_tensor(
                        prod[:, t * 6 + i, :].rearrange(
                            "p (j f h) -> p j f h", f=32, h=4),
                        al[:, t, i * 24:(i + 1) * 24]
                        .rearrange("p (j h) -> p j h", h=4)
                        .unsqueeze(2).broadcast_to([128, 6, 32, 4]),
                        hbL[:, t, :].rearrange("p (j f h) -> p j f h",
                                               f=32, h=4),
                        OP.mult,
                    )

            def tail_half(half):
                th0 = 2 * half
                cs = slice(th0 * 6, (th0 + 2) * 6)
                softmax_tile(th0)
                softmax_tile(th0 + 1)
                nc.vector.tensor_add(f1[:, cs, :], prod[:, cs, 0:384],
                                     prod[:, cs, 384:768])
                nc.vector.tensor_add(f2[:, cs, :], f1[:, cs, 0:128],
                                     f1[:, cs, 128:256])
                nc.vector.tensor_add(attn[:, cs, :], f2[:, cs, :],
                                     f1[:, cs, 256:384])
                # elu(x)+1 = exp(min(x,0)) + relu(x)  (-1 folded into c2);
                # min/relu at 4x tensor_scalar rate, exp on scalar.
                nc.vector.tensor_scalar_min(minat[:, cs, :], attn[:, cs, :],
                                            0.0)
                nc.scalar.activation(relux[:, cs, :], attn[:, cs, :],
                                     ACT.Relu)

            def tail_b(half):
                cs = slice(2 * half * 6, (2 * half + 2) * 6)
                for t in (2 * half, 2 * half + 1):
                    c6 = slice(t * 6, (t + 1) * 6)
                    nc.scalar.activation(expm[:, c6, :], minat[:, c6, :],
                                         ACT.Exp)
                    nc.vector.tensor_add(v1[:, c6, :], expm[:, c6, :],
                                         relux[:, c6, :])
                nc.vector.tensor_tensor(
                    vw[:, cs, :], v1[:, cs, :],
                    w2_sb[:, :].unsqueeze(1).broadcast_to([128, 12, 128]),
                    OP.mult)
                # fold W2-dot with cheap 2x adds, then a short 1x reduce
                nc.vector.tensor_add(vwa[:, cs, :], vw[:, cs, 0:64],
                                     vw[:, cs, 64:128])
                nc.vector.tensor_add(vwb[:, cs, :], vwa[:, cs, 0:32],
                                     vwa[:, cs, 32:64])
                nc.vector.tensor_reduce(h2[:, cs], vwb[:, cs, :], axis=AX.X,
                                        op=OP.add)

            def l2_half(half):
                # layer-2 GAT attention over the 6 nodes, per tile pair
                cs = slice(half * 12, (half + 1) * 12)
                nc.vector.tensor_scalar(h2c[:, cs], h2[:, cs], -c2, None,
                                        OP.add)
                nc.vector.tensor_scalar(h2s[:, cs], h2[:, cs], as2,
                                        -c2 * (as2 + ad2), OP.mult, OP.add)
                nc.vector.tensor_scalar(h2d[:, cs], h2[:, cs], ad2, None,
                                        OP.mult)
                h2dv = h2d[:, cs].rearrange("p (t i) -> p t i", t=2)
                h2sv = h2s[:, cs].rearrange("p (t j) -> p t j", t=2)
                h2cv = h2c[:, cs].rearrange("p (t j) -> p t j", t=2)
                es = slice(half * 72, (half + 1) * 72)
                nc.vector.tensor_tensor(
                    e2_raw[:, es].rearrange("p (t i j) -> p t i j", t=2, j=6),
                    h2dv.unsqueeze(3).broadcast_to([128, 2, 6, 6]),
                    h2sv.unsqueeze(2).broadcast_to([128, 2, 6, 6]),
                    OP.add,
                )
                nc.vector.scalar_tensor_tensor(
                    e2_lr[:, es], e2_raw[:, es], LRELU_SLOPE, e2_raw[:, es],
                    OP.mult, OP.max)
                nc.scalar.activation(E2[:, es], e2_lr[:, es], ACT.Exp)
                nc.vector.tensor_reduce(
                    Z2[:, cs],
                    E2[:, es].rearrange("p (ti j) -> p ti j", j=6),
                    axis=AX.X, op=OP.add)
                nc.vector.reciprocal_approx_fast(rZ2[:, cs], Z2[:, cs])
                nc.vector.tensor_tensor(
                    P2[:, es].rearrange("p (t i j) -> p t i j", t=2, j=6),
                    E2[:, es].rearrange("p (t i j) -> p t i j", t=2, j=6),
                    h2cv.unsqueeze(2).broadcast_to([128, 2, 6, 6]),
                    OP.mult,
                )
                nc.vector.tensor_reduce(
                    S2[:, cs],
                    P2[:, es].rearrange("p (ti j) -> p ti j", j=6),
                    axis=AX.X, op=OP.add,
                )
                nc.vector.tensor_mul(out_sb[:, cs], S2[:, cs], rZ2[:, cs])
                if b2 != 0.0:
                    nc.vector.tensor_scalar(out_sb[:, cs], out_sb[:, cs], b2,
                                            None, OP.add)
                nc.sync.dma_start(
                    out=bass.AP(tensor=out_ext, offset=half * 2 * 128 * K,
                                ap=[[K, 128], [128 * K, 2], [1, K]]),
                    in_=out_sb[:, cs].rearrange("p (t k) -> p t k", t=2))

            # emission order = per-engine program order: keep each half's
            # tail right after its two tiles so no engine's queue blocks
            # half-0 work behind tile-2/3 dependencies.
            pe_pair(0)
            tail_half(0)
            pe_pair(2)
            tail_b(0)
            l2_half(0)
            tail_half(1)
            tail_b(1)
            l2_half(1)

    nc.finalize()
    return nc


def prep_shared(inp):
    """Host-side layout prep shared across cores (index math / weight
    folding / dtype casts only -- all tensor FLOPs on the batch stay on
    device)."""
    bf = ml_dtypes.bfloat16
    node = np.asarray(inp["node_embeddings"], np.float32).astype(bf)
    sub = np.asarray(inp["substation_embeddings"], np.float32).astype(bf)
    elem = np.asarray(inp["sub_id_to_elem_id"], np.int64)
    tbl = np.empty((B, S, RB, H), bf)
    tbl[:, :, 0:K, :] = node[:, elem.reshape(-1), :].reshape(B, S, K, H)
    tbl[:, :, 6, :] = sub
    sub_idx = np.asarray(inp["sub_choice"], np.int64).reshape(B)
    obs = np.asarray(inp["org_obs"], np.float32)

    W1 = np.asarray(inp["W1"], np.float32)
    W1a, W1b, W1c = W1[0:H], W1[H:2 * H], W1[2 * H:3 * H]
    Wp = np.asarray(inp["W_proj"], np.float32)
    wfold = np.zeros((OBS_PAD, H), np.float32)
    wfold[:OBS] = Wp @ W1a
    wfold4 = wfold.reshape(4, 128, H).transpose(1, 0, 2).copy()
    bias_h = (np.asarray(inp["b1"], np.float32)
              + np.asarray(inp["b_proj"], np.float32) @ W1a)

    a_src1 = np.asarray(inp["a_src1"], np.float32)
    a_dst1 = np.asarray(inp["a_dst1"], np.float32)
    asrc_m = np.zeros((H, 8), np.float32)
    for h in range(HEADS):
        asrc_m[h * FH:(h + 1) * FH, h] = a_src1[h]
        asrc_m[h * FH:(h + 1) * FH, 4 + h] = a_dst1[h]
    # first 128 cols: permutation (h,f)->(f,h) so the transpose emits the
    # apply-friendly layout directly
    perm = np.zeros((H, H), np.float32)
    for h in range(HEADS):
        for f in range(FH):
            perm[h * FH + f, f * HEADS + h] = 1.0
    trident = np.concatenate([perm, asrc_m], axis=1)

    # e must be computed from h WITHOUT b1 (but WITH the obs-projection
    # bias, which is part of the reference h) -> correct only for b1.
    bh = np.asarray(inp["b1"], np.float32).reshape(HEADS, FH)
    cek = np.zeros((K, 8), np.float32)
    cek[:, 0:4] = (bh * a_src1).sum(-1)[None, :]
    cek[:, 4:8] = (bh * a_dst1).sum(-1)[None, :]

    shared = {
        "tbl": tbl,
        "sub_idx": sub_idx,
        "obs": obs,
        "wfold": wfold4.astype(bf),
        "w1bc": np.stack([W1b, W1c], axis=1).astype(bf),
        "trident": trident.astype(bf),
        "biash": bias_h.reshape(H, 1).astype(np.float32),
        "ce": cek.reshape(1, 48).astype(np.float32),
        # W2 permuted to the (f,h) feature order used by the apply layout
        "w2r": np.asarray(inp["W2"], np.float32).reshape(HEADS, FH).T
               .reshape(1, H).astype(bf),
        "identd": np.eye(H, dtype=np.float32).astype(bf),
    }
    return shared


def prep_core_inputs(core, shared):
    bf = ml_dtypes.bfloat16
    s = slice(core * BS, (core + 1) * BS)

    obs_T = np.zeros((OBS_PAD, BS), np.float32)
    obs_T[:OBS, :] = shared["obs"][s].T
    obs_T4 = obs_T.reshape(4, 128, BS).transpose(1, 0, 2).copy()

    sub_idx = shared["sub_idx"][s]
    idx_cols = []
    for t in range(NT):
        bl = np.arange(128) + t * 128
        v = (bl * S + sub_idx[bl]).astype(np.int16)
        blk = v.reshape(8, 16).T
        idx_cols.append(np.tile(blk, (8, 1)))
    idx2 = np.ascontiguousarray(np.concatenate(idx_cols, axis=1))

    return {
        "table7": shared["tbl"][s].reshape(BS * S, RB * H),
        "idx2": idx2,
        "obs_T": obs_T4.astype(bf),
        "wfold": shared["wfold"],
        "w1bc": shared["w1bc"],
        "trident": shared["trident"],
        "biash": shared["biash"],
        "ce": shared["ce"],
        "w2r": shared["w2r"],
        "identd": shared["identd"],
    }


_GRAPH_CACHE = {}
LAST_RESULTS = None


def kernel(**inputs):
    inp = {k: np.asarray(v) for k, v in inputs.items()}
    W2 = np.asarray(inp["W2"], np.float32)
    shared = prep_shared(inp)
    scalars = {
        "a_src2": float(np.asarray(inp["a_src2"]).reshape(-1)[0]),
        "a_dst2": float(np.asarray(inp["a_dst2"]).reshape(-1)[0]),
        "b2": float(np.asarray(inp["b2"]).reshape(-1)[0]),
        "c2": float(W2.sum()),
        "ce_nz": bool(np.any(np.abs(shared["ce"]) > 0)),
    }
    key = tuple(sorted(scalars.items()))
    if key not in _GRAPH_CACHE:
        _GRAPH_CACHE[key] = build_graph(scalars)
    nc = _GRAPH_CACHE[key]

    in_maps = [prep_core_inputs(c, shared) for c in range(NCORES)]
    res = run_bass_kernel_spmd(nc, in_maps, core_ids=list(range(NCORES)))
    global LAST_RESULTS
    LAST_RESULTS = res
    out = np.concatenate([res.results[c]["out"] for c in range(NCORES)], axis=0)
    return out.reshape(B, K, 1).astype(np.float32)


if __name__ == "__main__":
    g = build_graph({"a_src2": 0.01, "a_dst2": 0.02, "b2": 0.0, "c2": 0.1,
                     "ce_nz": False})
    print("graph built ok")
